# revision 2
# baseline (speedup 1.0000x reference)
"""BartAttention (focused-attention) Trainium2 kernel, v2.

Design (per core: batch b, head-group g of 4 heads = 2 pairs j=0,1):
  - qkv projections bf16 (q,k pre-scaled x16/x8 into fp8 storage for scores)
  - scores via fp8e4m3 DoubleRow matmuls: hd=64 split into 2x32 on
    partitions 0-31 (reshuffle DMA), sc[s,t] = 128*q.k in PSUM f32
  - exp on ACT only: e = exp(sc/128) [128,1024] tiles (2 heads) - ACT is
    the bottleneck engine (~133us busy), everything else hides under it
  - e = e*fT in-place on DVE (4x mode)
  - PV e-stationary: acc[t,65] += e_block.T @ [v|1], N=65 matmuls; col 64
    gives rowsum; per-partition normalization via reciprocal+tensor_scalar
  - po transposed to [r,t] via DMA-transpose (no PE/PSUM involved)
  - out-proj per (t-block, d-chunk) accumulating over both head pairs,
    PSUM -> SBUF copy on gpsimd -> DMA out
  - schedule: u00 live; u01/u10 exp'd during QKV phase with deferred PV
    (e backlog in SBUF) to keep ACT fed; phase B replays PV one unit
    behind the fresh scores stream.

Host sums the 4 partial out-projections per batch and adds bo.
"""

import numpy as np
import ml_dtypes

import concourse.bass as bass
import concourse.bacc as bacc
import concourse.mybir as mybir
from concourse.tile import TileContext
from concourse.bass_utils import run_bass_kernel_spmd

BF16 = mybir.dt.bfloat16
F32 = mybir.dt.float32
F8 = mybir.dt.float8e4
AF = mybir.ActivationFunctionType
DR = mybir.MatmulPerfMode.DoubleRow

B, T, D = 2, 2048, 1024
H, HD = 16, 64
HG = 4               # heads per core
R = HG * HD          # 256 rows per core
SCALING = HD ** -0.5
N_CORES = 8

P = 128
KT = D // P          # 8 k-tiles for QKV contraction
NCH = T // 512       # 4 t-chunks
ST = T // P          # 16 s-tiles

QSC, KSC = 16.0, 8.0         # fp8 pre-scales for q and k
EXP_SCALE = 1.0 / (QSC * KSC)


def build_bass():
    nc = bacc.Bacc()

    hT_d = nc.declare_dram_parameter("hT", [D, T], BF16, isOutput=False)
    fT_d = nc.declare_dram_parameter("fT", [T, T], BF16, isOutput=False)
    wqT_d = nc.declare_dram_parameter("wqT", [D, R], BF16, isOutput=False)
    wkT_d = nc.declare_dram_parameter("wkT", [D, R], BF16, isOutput=False)
    wvT_d = nc.declare_dram_parameter("wvT", [D, R], BF16, isOutput=False)
    woT_d = nc.declare_dram_parameter("woT", [R, D], BF16, isOutput=False)
    bq_d = nc.declare_dram_parameter("bq", [R, 1], F32, isOutput=False)
    bk_d = nc.declare_dram_parameter("bk", [R, 1], F32, isOutput=False)
    bv_d = nc.declare_dram_parameter("bv", [1, R], BF16, isOutput=False)
    id_d = nc.declare_dram_parameter("ident", [P, P], BF16, isOutput=False)
    out_d = nc.declare_dram_parameter("out_partial", [T, D], F32, isOutput=True)

    hT_r = hT_d.rearrange("(k p) t -> p k t", p=P)
    fT_r = fT_d.rearrange("(s p) t -> p s t", p=P)

    with TileContext(nc) as tc:
        with (
            nc.allow_low_precision(reason="bf16/fp8 pipeline is intentional"),
            tc.tile_pool(name="sb", bufs=1) as sb,
            tc.tile_pool(name="ps", bufs=1, space="PSUM") as ps,
        ):
            # ---- persistent SBUF ----
            wq = sb.tile([P, KT, R], BF16)
            wk = sb.tile([P, KT, R], BF16)
            wv = sb.tile([P, KT, R], BF16)
            wo = sb.tile([P, 2, D], BF16)
            bq = sb.tile([P, 2], F32)
            bk = sb.tile([P, 2], F32)
            bv = sb.tile([1, R], BF16)
            ident = sb.tile([P, P], BF16)
            ones_r = sb.tile([1, P], BF16)
            q8s = sb.tile([32, 2, HG, T], F8)   # [p32, half, head, t]
            k8s = sb.tile([32, 2, HG, T], F8)
            vsb = sb.tile([P, ST, HG, HD + 1], BF16)
            qT0 = sb.tile([P, 2, 512], BF16)    # chunk-0 q bf16 (ramp path)
            kT0 = sb.tile([P, 2, 512], BF16)
            po = sb.tile([P, 2, T], BF16)       # out-proj lhsT [r, (tch c t)]

            nc.vector.memset(ones_r[:], 1.0)
            nc.vector.memset(vsb[:, :, :, HD:HD + 1], 1.0)

            # PE warm-up: burn the p-state ramp on junk matmuls during DMA
            for i in range(24):
                wrm = ps.tile([P, P], F32, tag="sc", bufs=2, name=f"warm{i}")
                nc.tensor.matmul(wrm[:], ones_r[:], ones_r[:],
                                 start=True, stop=True)

            # ---- initial DMAs ----
            nc.sync.dma_start(wq[:], wqT_d.rearrange("(k p) r -> p k r", p=P))
            ht = {}

            def dma_ht(n):
                t = sb.tile([P, KT, 512], BF16, tag="ht", bufs=3, name=f"ht{n}")
                nsl = slice(n * 512, (n + 1) * 512)
                nc.sync.dma_start(t[:, 0:4, :], hT_r[:, 0:4, nsl])
                nc.sync.dma_start(t[:, 4:8, :], hT_r[:, 4:8, nsl])
                ht[n] = t

            ht0 = sb.tile([P, KT, 512], BF16, tag="ht", bufs=3, name="ht0")
            nc.sync.dma_start(ht0[:, 0:4, :], hT_r[:, 0:4, 0:512])
            nc.sync.dma_start(wk[:], wkT_d.rearrange("(k p) r -> p k r", p=P))
            nc.sync.dma_start(ht0[:, 4:8, :], hT_r[:, 4:8, 0:512])
            ht[0] = ht0
            nc.sync.dma_start(bq[:], bq_d.rearrange("(m p) one -> p (m one)", p=P))
            nc.sync.dma_start(bk[:], bk_d.rearrange("(m p) one -> p (m one)", p=P))

            fts = {}

            def dma_ft(tch, g):
                t = sb.tile([P, 4, 512], BF16, tag="ft", bufs=8,
                            name=f"ft{tch}g{g}")
                nc.sync.dma_start(
                    t[:], fT_r[:, 4 * g:4 * g + 4, tch * 512:(tch + 1) * 512]
                )
                fts[(tch, g)] = t

            # ---- helpers ----
            st8s = {}

            def qk_part(n, tens, m):
                """q or k projection matmuls + bias for chunk n, m-block."""
                w_sb, b_sb = (wq, bq) if tens == 0 else (wk, bk)
                nm = f"{'qk'[tens]}{n}m{m}"
                acc = ps.tile([P, 512], F32, tag="aux", bufs=2, name=f"a{nm}")
                for kk in range(KT):
                    nc.tensor.matmul(
                        acc[:], w_sb[:, kk, m * P:(m + 1) * P],
                        ht[n][:, kk, :],
                        start=(kk == 0), stop=(kk == KT - 1),
                    )
                if n == 0:
                    dst = qT0 if tens == 0 else kT0
                    nc.vector.tensor_scalar_add(dst[:, m, :], acc[:],
                                                b_sb[:, m:m + 1])
                    return
                if (tens, n) not in st8s:
                    st8s[(tens, n)] = sb.tile([P, 2, 512], F8, tag="st8",
                                              bufs=4, name=f"s{'qk'[tens]}{n}")
                nc.vector.tensor_scalar_add(st8s[(tens, n)][:, m, :], acc[:],
                                            b_sb[:, m:m + 1])

            def qk_cast0(tens, m):
                if (tens, 0) not in st8s:
                    st8s[(tens, 0)] = sb.tile([P, 2, 512], F8, tag="st8",
                                              bufs=4, name=f"s{'qk'[tens]}0")
                src_t = qT0 if tens == 0 else kT0
                nc.vector.tensor_copy(st8s[(tens, 0)][:, m, :], src_t[:, m, :])

            def qk_resh(n, tens, m=None):
                """Reshuffle chunk n into the [32, half, head, t] hd-split
                layout. m=None: both m-blocks in 4 DMAs of [32, 2, 512]
                (m via stride-2 head dim); m=int: that m-block only."""
                dst = q8s if tens == 0 else k8s
                st8 = st8s[(tens, n)]
                nsl = slice(n * 512, (n + 1) * 512)
                for half in range(2):
                    for hm in range(2):
                        src_p = slice(64 * hm + 32 * half, 64 * hm + 32 * half + 32)
                        if m is None:
                            nc.sync.dma_start(dst[:, half, hm::2, nsl],
                                              st8[src_p, :, :])
                        else:
                            nc.sync.dma_start(dst[:, half, 2 * m + hm, nsl],
                                              st8[src_p, m, :])

            def v_stile(s):
                acc = ps.tile([P, R], F32, tag="aux", bufs=2, name=f"vacc{s}")
                for kk in range(KT):
                    nc.tensor.matmul(
                        acc[:], ht[s // 4][:, kk, (s % 4) * P:(s % 4 + 1) * P],
                        wv[:, kk, :], start=(kk == 0), stop=False,
                    )
                nc.tensor.matmul(acc[:], ones_r[:], bv[:], start=False, stop=True)
                nc.vector.tensor_copy(
                    vsb[:, s, :, 0:HD],
                    acc[:].rearrange("p (h d) -> p h d", h=HG),
                )

            ebank = {}    # u -> {st: e_tile} pending PV
            accs = {}     # u -> (acc_a, acc_b)
            poTs = {}     # (u, c) -> poT tile

            def scores_step(u, st):
                tch, j = u
                sc = ps.tile([P, 1024], F32, tag="sc", bufs=2,
                             name=f"sc{tch}{j}_{st}")
                tsl = slice(tch * 512, (tch + 1) * 512)
                ssl = slice(st * P, (st + 1) * P)
                for a in range(2):
                    h = 2 * j + a
                    if tch == 0 and st < 4:
                        rows = slice(a * HD, (a + 1) * HD)
                        nc.tensor.matmul(
                            sc[:, a * 512:(a + 1) * 512],
                            kT0[rows, j, st * P:(st + 1) * P],
                            qT0[rows, j, :],
                            start=True, stop=True,
                        )
                    else:
                        nc.tensor.matmul(
                            sc[:, a * 512:(a + 1) * 512],
                            k8s[:, :, h, ssl], q8s[:, :, h, tsl],
                            start=True, stop=True, perf_mode=DR,
                        )
                e = sb.tile([P, 1024], BF16, tag="e", bufs=32,
                            name=f"e{tch}{j}_{st}")
                nc.scalar.activation(e[:], sc[:], AF.Exp, scale=EXP_SCALE)
                ftt = fts[(tch, st // 4)]
                eng = nc.gpsimd if st in (3, 9, 14) else nc.vector
                for a in range(2):
                    half = slice(a * 512, (a + 1) * 512)
                    eng.tensor_mul(e[:, half], e[:, half],
                                   ftt[:, st % 4, :])
                ebank[u][st] = e

            def alloc_accs(u):
                accs[u] = tuple(
                    ps.tile([P, 4, HD + 1], F32, tag="uacc", bufs=2,
                            name=f"acc{u[0]}{u[1]}{a}")
                    for a in range(2)
                )

            def pv_step(u, st, first=None, last=None):
                tch, j = u
                first = (st == 0) if first is None else first
                last = (st == ST - 1) if last is None else last
                e = ebank[u].pop(st)
                for a in range(2):
                    acc = accs[u][a]
                    for c in range(4):
                        nc.tensor.matmul(
                            acc[:, c, :],
                            e[:, a * 512 + c * P:a * 512 + (c + 1) * P],
                            vsb[:, st, 2 * j + a, :],
                            start=(first and c == 0),
                            stop=(last and c == 3),
                            skip_group_check=True,
                        )

            def norm_unit(u, tail=False):
                for a in range(2):
                    acc = accs[u][a]
                    rc = sb.tile([P, 4, 1], F32, tag="rc", bufs=4,
                                 name=f"rc{u[0]}{u[1]}{a}")
                    nc.vector.reciprocal(rc[:], acc[:, :, HD:HD + 1])
                    for c in range(4):
                        if (u, c) not in poTs:
                            poTs[(u, c)] = sb.tile(
                                [P, P], BF16, tag="pt", bufs=8,
                                name=f"pt{u[0]}{u[1]}{c}")
                        if tail and a == 1:
                            nc.scalar.mul(
                                poTs[(u, c)][:, a * HD:(a + 1) * HD],
                                acc[:, c, 0:HD], rc[:, c, :],
                            )
                        else:
                            nc.vector.tensor_scalar_mul(
                                poTs[(u, c)][:, a * HD:(a + 1) * HD],
                                acc[:, c, 0:HD], rc[:, c, :],
                            )

            def tp_one(u, c):
                tch, j = u
                tpp = ps.tile([P, P], BF16, tag="aux", bufs=2,
                              name=f"tpp{tch}{j}{c}")
                nc.tensor.transpose(tpp[:], poTs.pop((u, c))[:], ident[:])
                nc.vector.tensor_copy(
                    po[:, j, tch * 512 + c * P: tch * 512 + (c + 1) * P],
                    tpp[:],
                )

            def tp_unit(u):
                for c in range(4):
                    tp_one(u, c)

            def fin_one(tch, c, tag="aux"):
                """Out-proj for one t-block: both 512-wide d-chunks, one store."""
                tsl = slice(tch * 512 + c * P, tch * 512 + (c + 1) * P)
                for dch in range(2):
                    fptag = tag if dch == 0 else ("uacc" if tch == 3 else tag)
                    fp = ps.tile([P, 512], F32, tag=fptag, bufs=2,
                                 name=f"fp{tch}{c}{dch}")
                    dsl = slice(dch * 512, (dch + 1) * 512)
                    for j in range(2):
                        nc.tensor.matmul(fp[:], po[:, j, tsl], wo[:, j, dsl],
                                         start=(j == 0), stop=(j == 1))
                    fo = sb.tile([P, 512], F32, tag="fo", bufs=4,
                                 name=f"fo{tch}{c}{dch}")
                    nc.vector.tensor_copy(fo[:], fp[:])
                    nc.sync.dma_start(out_d[tsl, dsl], fo[:])

            # ================= PHASE A: QKV chunks + u00/u01/u10 scores ======
            u00, u01, u10, u11 = (0, 0), (0, 1), (1, 0), (1, 1)
            for u in (u00, u01, u10, u11):
                ebank[u] = {}
            alloc_accs(u00)

            # chunk 0
            qk_part(0, 0, 0)          # q0 m0 -> qT0
            qk_part(0, 1, 0)          # k0 m0 -> kT0
            dma_ft(0, 0)
            dma_ht(1)
            nc.sync.dma_start(bv[:], bv_d[:])
            nc.sync.dma_start(ident[:], id_d[:])
            for st in range(4):
                scores_step(u00, st)
            qk_cast0(0, 0)
            qk_resh(0, 0, 0)
            qk_cast0(1, 0)
            qk_resh(0, 1, 0)
            qk_part(0, 0, 1)
            qk_part(0, 1, 1)
            dma_ft(0, 1)
            for st in range(4):
                scores_step(u01, st)
            qk_cast0(0, 1)
            qk_resh(0, 0, 1)
            qk_cast0(1, 1)
            qk_resh(0, 1, 1)
            nc.sync.dma_start(wv[:], wvT_d.rearrange("(k p) r -> p k r", p=P))

            # chunks 1-3, serpentine: each chunk's k m-blocks are emitted
            # while the previous groups' exps are still queued, so the
            # bias+reshuffle chain hides; q(n>=1) and v are off-critical.
            qk_part(1, 1, 0)
            qk_resh(1, 1, 0)
            for n in range(1, 4):
                for st in range(4 * n, 4 * n + 4):
                    scores_step(u00, st)
                qk_part(n, 1, 1)
                qk_resh(n, 1, 1)
                qk_part(n, 0, 0)
                qk_part(n, 0, 1)
                qk_resh(n, 0)         # q merged, off critical path
                if n < 3:
                    dma_ht(n + 1)
                    dma_ft(0, n + 1)
                dma_ft(1, n - 1)
                for st in range(4 * n, 4 * n + 4):
                    scores_step(u01, st)
                for st in range(4 * n - 4, 4 * n):
                    scores_step(u10, st)
                for s in range(4 * n - 4, 4 * n):
                    v_stile(s)
                for st in range(4 * n - 4, 4 * n):
                    pv_step(u00, st)
                if n < 3:
                    qk_part(n + 1, 1, 0)
                    qk_resh(n + 1, 1, 0)
                if n == 3:
                    dma_ft(1, 3)

            # phase-A tail: last v group + u00 finish
            for s in range(12, 16):
                v_stile(s)
            for st in range(12, 16):
                pv_step(u00, st)
            nc.sync.dma_start(wo[:], woT_d.rearrange("(m p) d -> p m d", p=P))

            norm_unit(u00)
            tp_unit(u00)

            # ================= PHASE B: weave ===============================
            fresh = [u11, (2, 0), (2, 1), (3, 0), (3, 1)]
            work = [(u10, st) for st in range(12, 16)]
            for w in fresh:
                ebank.setdefault(w, {})
                work += [(w, st) for st in range(ST)]

            # replay queue: (unit, st) in replay order; fresh units appended
            # as their scores complete
            RORD = [0, 1, 2, 4, 5, 6, 7, 8, 10, 11, 12, 13, 3, 9, 14, 15]
            replayq = [(u01, st) for st in RORD]
            replayq += [(u10, st) for st in RORD]
            for w in fresh:
                replayq += [(w, st) for st in RORD]

            # fins become available per tch once both units' tps are done
            finq = []
            tp_done = {u00: True}
            fins_emitted = set()

            ft_sched = {
                (fresh[0], 2): (2, 0), (fresh[0], 6): (2, 1),
                (fresh[0], 10): (2, 2), (fresh[0], 14): (2, 3),
                ((2, 0), 2): (3, 0), ((2, 0), 6): (3, 1),
                ((2, 0), 10): (3, 2), ((2, 0), 14): (3, 3),
            }

            rpi = 0
            sci = 0
            cool = 0
            for (w, st) in work:
                scores_step(w, st)
                sci += 1
                # splice replay PV steps; a step can only replay once its e
                # tile is produced.  After a unit finishes (norm+transpose),
                # pause splicing so the next unit's first PV does not block
                # the PE pipeline while PSUM accumulators drain.
                budget = 2
                if cool > 0:
                    cool -= 1
                    budget = 0
                while budget > 0 and rpi < len(replayq):
                    ru, rst = replayq[rpi]
                    if rst not in ebank.get(ru, {}):
                        break  # not scored yet
                    if rst == 0:
                        alloc_accs(ru)
                    pv_step(ru, rst)
                    budget -= 1
                    rpi += 1
                    if rst == ST - 1:
                        norm_unit(ru)
                        tp_unit(ru)
                        tp_done[ru] = True
                        cool = 1
                        tch = ru[0]
                        other = (tch, 1 - ru[1])
                        if tp_done.get(other) and tch not in fins_emitted:
                            fins_emitted.add(tch)
                            finq += [(tch, c) for c in range(4)]
                        break
                # one fin (both d-chunks) every few scores steps
                if finq and sci % 4 == 0:
                    ftch, fc = finq.pop(0)
                    fin_one(ftch, fc, tag="uacc" if ftch == 3 else "aux")
                if (w, st) in ft_sched:
                    dma_ft(*ft_sched[(w, st)])

            # ---- tail: drain remaining replays, then c-pipelined tp+fin ----
            while rpi < len(replayq):
                ru, rst = replayq[rpi]
                if rst == 0:
                    alloc_accs(ru)
                pv_step(ru, rst)
                rpi += 1
                if rst == ST - 1:
                    norm_unit(ru, tail=True)
                    tch = ru[0]
                    other = (tch, 1 - ru[1])
                    if tp_done.get(other) and tch not in fins_emitted:
                        fins_emitted.add(tch)
                        tp_done[ru] = True
                        for c in range(4):
                            tp_one(ru, c)
                            fin_one(tch, c, tag="uacc")
                    else:
                        tp_unit(ru)
                        tp_done[ru] = True
                while finq:
                    ftch, fc = finq.pop(0)
                    fin_one(ftch, fc, tag="uacc" if ftch == 3 else "aux")
            while finq:
                ftch, fc = finq.pop(0)
                fin_one(ftch, fc, tag="uacc" if ftch == 3 else "aux")

    return nc


_NC = None
_LAST_RESULT = None


def _get_nc():
    global _NC
    if _NC is None:
        _NC = build_bass()
        if not _NC.is_finalized():
            _NC.finalize()
    return _NC


def kernel(hidden_states, focused_attention, Wq, bq, Wk, bk, Wv, bv, Wo, bo):
    bf = ml_dtypes.bfloat16
    hT = [np.ascontiguousarray(hidden_states[b].T).astype(bf) for b in range(B)]
    fT = [np.ascontiguousarray(focused_attention[b].T).astype(bf) for b in range(B)]

    in_maps = []
    for c in range(N_CORES):
        b, g = divmod(c, 4)
        rows = slice(g * R, (g + 1) * R)
        in_maps.append({
            "hT": hT[b],
            "fT": fT[b],
            "wqT": np.ascontiguousarray((Wq[rows] * (SCALING * QSC)).T).astype(bf),
            "wkT": np.ascontiguousarray((Wk[rows] * KSC).T).astype(bf),
            "wvT": np.ascontiguousarray(Wv[rows].T).astype(bf),
            "woT": np.ascontiguousarray(Wo[:, rows].T).astype(bf),
            "bq": np.ascontiguousarray(
                (bq[rows] * (SCALING * QSC))[:, None]).astype(np.float32),
            "bk": np.ascontiguousarray((bk[rows] * KSC)[:, None]).astype(np.float32),
            "bv": np.ascontiguousarray(bv[rows][None, :]).astype(bf),
            "ident": np.eye(P, dtype=bf),
        })

    res = run_bass_kernel_spmd(_get_nc(), in_maps, list(range(N_CORES)))
    global _LAST_RESULT
    _LAST_RESULT = res
    out = np.zeros((B, T, D), dtype=np.float32)
    for c in range(N_CORES):
        out[c // 4] += res.results[c]["out_partial"]
    out += np.asarray(bo, dtype=np.float32)[None, None, :]
    return out


# revision 3
# speedup vs baseline: 1.0005x; 1.0005x over previous
"""BartAttention (focused-attention variant) Trainium2 Bass kernel, v2.

Problem (hardcoded): B=2, T=2048, D=1024, H=16 heads, hd=64.
  q = (h @ Wq.T + bq) * hd**-0.5 ; k = h @ Wk.T + bk ; v = h @ Wv.T + bv
  scores = q @ k.T per head ; e = f * exp(scores) ; attn = e / rowsum(e)
  out = (attn @ v) @ Wo.T + bo

Sharding over 8 cores: batch (2) x head-group (4 groups of 4 heads); host
sums the 4 partial out-projections per batch and adds bo.

Per-core design (4 heads = 2 pairs j=0,1; ACT-exp is the bottleneck engine
at ~134us busy, everything else is scheduled to hide under it):
  - q/k projections bf16; q,k pre-scaled x16/x8 on the host and stored as
    fp8e4m3 in an hd-split [32 partitions, half, head, t] layout via
    SBUF->SBUF reshuffle DMAs
  - scores: fp8 DoubleRow matmuls (two 32-row contraction tiles per
    instruction -> 256 cycles per [128,512] block, 2x over bf16);
    sc = 128*q.k in PSUM f32; the first 4 s-tile groups of the (0,j) units
    use a bf16 path instead so the exp stream starts before any reshuffle
  - exp: ACT only, e = exp(sc/128) on [128,1024] tiles (2 heads);
    e *= fT in place on DVE (a few steps per unit go to GPSIMD to keep
    DVE under ACT)
  - PV e-stationary: acc[t-block, 65] += e_block.T @ [v|1]; N=65 matmuls;
    column 64 accumulates rowsum(e), so normalization is a per-partition
    reciprocal + tensor_scalar multiply (no broadcast matmuls)
  - po blocks transposed to [r, t] with PE transpose + DVE copy;
    out-proj per (t-block, d-chunk) accumulates both head pairs, DVE/ACT
    copies PSUM->SBUF, DMA out
  - schedule: QKV chunks serpentined with the scores of units (0,0), (0,1)
    and (1,0) (PV deferred into an SBUF e-backlog) so ACT never starves in
    phase A; phase B replays each unit's PV one unit behind the fresh
    scores stream, with out-projections and transposes woven between.
"""

import numpy as np
import ml_dtypes

import concourse.bass as bass
import concourse.bacc as bacc
import concourse.mybir as mybir
from concourse.tile import TileContext
from concourse.bass_utils import run_bass_kernel_spmd

BF16 = mybir.dt.bfloat16
F32 = mybir.dt.float32
F8 = mybir.dt.float8e4
AF = mybir.ActivationFunctionType
DR = mybir.MatmulPerfMode.DoubleRow

B, T, D = 2, 2048, 1024
H, HD = 16, 64
HG = 4               # heads per core
R = HG * HD          # 256 rows per core
SCALING = HD ** -0.5
N_CORES = 8

P = 128
KT = D // P          # 8 k-tiles for QKV contraction
NCH = T // 512       # 4 t-chunks
ST = T // P          # 16 s-tiles

QSC, KSC = 16.0, 8.0         # fp8 pre-scales for q and k
EXP_SCALE = 1.0 / (QSC * KSC)


def build_bass():
    nc = bacc.Bacc()

    hT_d = nc.declare_dram_parameter("hT", [D, T], BF16, isOutput=False)
    fT_d = nc.declare_dram_parameter("fT", [T, T], BF16, isOutput=False)
    wqT_d = nc.declare_dram_parameter("wqT", [D, R], BF16, isOutput=False)
    wkT_d = nc.declare_dram_parameter("wkT", [D, R], BF16, isOutput=False)
    wvT_d = nc.declare_dram_parameter("wvT", [D, R], BF16, isOutput=False)
    woT_d = nc.declare_dram_parameter("woT", [R, D], BF16, isOutput=False)
    bq_d = nc.declare_dram_parameter("bq", [R, 1], F32, isOutput=False)
    bk_d = nc.declare_dram_parameter("bk", [R, 1], F32, isOutput=False)
    bv_d = nc.declare_dram_parameter("bv", [1, R], BF16, isOutput=False)
    id_d = nc.declare_dram_parameter("ident", [P, P], BF16, isOutput=False)
    out_d = nc.declare_dram_parameter("out_partial", [T, D], F32, isOutput=True)

    hT_r = hT_d.rearrange("(k p) t -> p k t", p=P)
    fT_r = fT_d.rearrange("(s p) t -> p s t", p=P)

    with TileContext(nc) as tc:
        with (
            nc.allow_low_precision(reason="bf16/fp8 pipeline is intentional"),
            tc.tile_pool(name="sb", bufs=1) as sb,
            tc.tile_pool(name="ps", bufs=1, space="PSUM") as ps,
        ):
            # ---- persistent SBUF ----
            wq = sb.tile([P, KT, R], BF16)
            wk = sb.tile([P, KT, R], BF16)
            wv = sb.tile([P, KT, R], BF16)
            wo = sb.tile([P, 2, D], BF16)
            bq = sb.tile([P, 2], F32)
            bk = sb.tile([P, 2], F32)
            bv = sb.tile([1, R], BF16)
            ident = sb.tile([P, P], BF16)
            ones_r = sb.tile([1, P], BF16)
            q8s = sb.tile([32, 2, HG, T], F8)   # [p32, half, head, t]
            k8s = sb.tile([32, 2, HG, T], F8)
            vsb = sb.tile([P, ST, HG, HD + 1], BF16)
            qT0 = sb.tile([P, 2, 512], BF16)    # chunk-0 q bf16 (ramp path)
            kT0 = sb.tile([P, 2, 512], BF16)
            po = sb.tile([P, 2, T], BF16)       # out-proj lhsT [r, (tch c t)]

            nc.vector.memset(ones_r[:], 1.0)
            nc.vector.memset(vsb[:, :, :, HD:HD + 1], 1.0)

            # PE warm-up: burn the p-state ramp on junk matmuls during DMA
            for i in range(24):
                wrm = ps.tile([P, P], F32, tag="sc", bufs=2, name=f"warm{i}")
                nc.tensor.matmul(wrm[:], ones_r[:], ones_r[:],
                                 start=True, stop=True)

            # ---- initial DMAs ----
            nc.sync.dma_start(wq[:], wqT_d.rearrange("(k p) r -> p k r", p=P))
            ht = {}

            def dma_ht(n):
                t = sb.tile([P, KT, 512], BF16, tag="ht", bufs=3, name=f"ht{n}")
                nsl = slice(n * 512, (n + 1) * 512)
                nc.sync.dma_start(t[:, 0:4, :], hT_r[:, 0:4, nsl])
                nc.sync.dma_start(t[:, 4:8, :], hT_r[:, 4:8, nsl])
                ht[n] = t

            ht0 = sb.tile([P, KT, 512], BF16, tag="ht", bufs=3, name="ht0")
            nc.sync.dma_start(ht0[:, 0:4, :], hT_r[:, 0:4, 0:512])
            nc.sync.dma_start(wk[:], wkT_d.rearrange("(k p) r -> p k r", p=P))
            nc.sync.dma_start(ht0[:, 4:8, :], hT_r[:, 4:8, 0:512])
            ht[0] = ht0
            nc.sync.dma_start(bq[:], bq_d.rearrange("(m p) one -> p (m one)", p=P))
            nc.sync.dma_start(bk[:], bk_d.rearrange("(m p) one -> p (m one)", p=P))

            fts = {}

            def dma_ft(tch, g):
                t = sb.tile([P, 4, 512], BF16, tag="ft", bufs=8,
                            name=f"ft{tch}g{g}")
                nc.sync.dma_start(
                    t[:], fT_r[:, 4 * g:4 * g + 4, tch * 512:(tch + 1) * 512]
                )
                fts[(tch, g)] = t

            # ---- helpers ----
            st8s = {}

            def qk_part(n, tens, m, cols=None):
                """q or k projection matmuls + bias for chunk n, m-block.
                cols: optional (lo, hi) sub-range of the 512 chunk columns."""
                w_sb, b_sb = (wq, bq) if tens == 0 else (wk, bk)
                lo, hi = cols if cols else (0, 512)
                nm = f"{'qk'[tens]}{n}m{m}c{lo}"
                acc = ps.tile([P, hi - lo], F32, tag="aux", bufs=2,
                              name=f"a{nm}")
                for kk in range(KT):
                    nc.tensor.matmul(
                        acc[:], w_sb[:, kk, m * P:(m + 1) * P],
                        ht[n][:, kk, lo:hi],
                        start=(kk == 0), stop=(kk == KT - 1),
                    )
                if n == 0:
                    dst = qT0 if tens == 0 else kT0
                    nc.vector.tensor_scalar_add(dst[:, m, lo:hi], acc[:],
                                                b_sb[:, m:m + 1])
                    return
                if (tens, n) not in st8s:
                    st8s[(tens, n)] = sb.tile([P, 2, 512], F8, tag="st8",
                                              bufs=4, name=f"s{'qk'[tens]}{n}")
                nc.vector.tensor_scalar_add(st8s[(tens, n)][:, m, lo:hi],
                                            acc[:], b_sb[:, m:m + 1])

            def qk_cast0(tens, m):
                if (tens, 0) not in st8s:
                    st8s[(tens, 0)] = sb.tile([P, 2, 512], F8, tag="st8",
                                              bufs=4, name=f"s{'qk'[tens]}0")
                src_t = qT0 if tens == 0 else kT0
                nc.vector.tensor_copy(st8s[(tens, 0)][:, m, :], src_t[:, m, :])

            def qk_resh(n, tens, m=None, cols=None):
                """Reshuffle chunk n into the [32, half, head, t] hd-split
                layout. m=None: both m-blocks in 4 DMAs of [32, 2, 512]
                (m via stride-2 head dim); m=int: that m-block only."""
                dst = q8s if tens == 0 else k8s
                st8 = st8s[(tens, n)]
                lo, hi = cols if cols else (0, 512)
                nsl = slice(n * 512 + lo, n * 512 + hi)
                for half in range(2):
                    for hm in range(2):
                        src_p = slice(64 * hm + 32 * half, 64 * hm + 32 * half + 32)
                        if m is None:
                            nc.sync.dma_start(dst[:, half, hm::2, nsl],
                                              st8[src_p, :, lo:hi])
                        else:
                            nc.sync.dma_start(dst[:, half, 2 * m + hm, nsl],
                                              st8[src_p, m, lo:hi])

            def v_stile(s):
                acc = ps.tile([P, R], F32, tag="aux", bufs=2, name=f"vacc{s}")
                for kk in range(KT):
                    nc.tensor.matmul(
                        acc[:], ht[s // 4][:, kk, (s % 4) * P:(s % 4 + 1) * P],
                        wv[:, kk, :], start=(kk == 0), stop=False,
                    )
                nc.tensor.matmul(acc[:], ones_r[:], bv[:], start=False, stop=True)
                nc.vector.tensor_copy(
                    vsb[:, s, :, 0:HD],
                    acc[:].rearrange("p (h d) -> p h d", h=HG),
                )

            ebank = {}    # u -> {st: e_tile} pending PV
            accs = {}     # u -> (acc_a, acc_b)
            poTs = {}     # (u, c) -> poT tile

            def scores_step(u, st):
                tch, j = u
                sc = ps.tile([P, 1024], F32, tag="sc", bufs=2,
                             name=f"sc{tch}{j}_{st}")
                tsl = slice(tch * 512, (tch + 1) * 512)
                ssl = slice(st * P, (st + 1) * P)
                for a in range(2):
                    h = 2 * j + a
                    if tch == 0 and st < 4:
                        rows = slice(a * HD, (a + 1) * HD)
                        nc.tensor.matmul(
                            sc[:, a * 512:(a + 1) * 512],
                            kT0[rows, j, st * P:(st + 1) * P],
                            qT0[rows, j, :],
                            start=True, stop=True,
                        )
                    else:
                        nc.tensor.matmul(
                            sc[:, a * 512:(a + 1) * 512],
                            k8s[:, :, h, ssl], q8s[:, :, h, tsl],
                            start=True, stop=True, perf_mode=DR,
                        )
                e = sb.tile([P, 1024], BF16, tag="e", bufs=32,
                            name=f"e{tch}{j}_{st}")
                nc.scalar.activation(e[:], sc[:], AF.Exp, scale=EXP_SCALE)
                ftt = fts[(tch, st // 4)]
                eng = nc.gpsimd if st in (3, 9, 14) else nc.vector
                for a in range(2):
                    half = slice(a * 512, (a + 1) * 512)
                    eng.tensor_mul(e[:, half], e[:, half],
                                   ftt[:, st % 4, :])
                ebank[u][st] = e

            def alloc_accs(u):
                accs[u] = tuple(
                    ps.tile([P, 4, HD + 1], F32, tag="uacc", bufs=2,
                            name=f"acc{u[0]}{u[1]}{a}")
                    for a in range(2)
                )

            def pv_step(u, st, first=None, last=None):
                tch, j = u
                first = (st == 0) if first is None else first
                last = (st == ST - 1) if last is None else last
                e = ebank[u].pop(st)
                for a in range(2):
                    acc = accs[u][a]
                    for c in range(4):
                        nc.tensor.matmul(
                            acc[:, c, :],
                            e[:, a * 512 + c * P:a * 512 + (c + 1) * P],
                            vsb[:, st, 2 * j + a, :],
                            start=(first and c == 0),
                            stop=(last and c == 3),
                            skip_group_check=True,
                        )

            def norm_unit(u, tail=False):
                for a in range(2):
                    acc = accs[u][a]
                    rc = sb.tile([P, 4, 1], F32, tag="rc", bufs=4,
                                 name=f"rc{u[0]}{u[1]}{a}")
                    nc.vector.reciprocal(rc[:], acc[:, :, HD:HD + 1])
                    for c in range(4):
                        if (u, c) not in poTs:
                            poTs[(u, c)] = sb.tile(
                                [P, P], BF16, tag="pt", bufs=8,
                                name=f"pt{u[0]}{u[1]}{c}")
                        if tail and a == 1:
                            nc.scalar.mul(
                                poTs[(u, c)][:, a * HD:(a + 1) * HD],
                                acc[:, c, 0:HD], rc[:, c, :],
                            )
                        else:
                            nc.vector.tensor_scalar_mul(
                                poTs[(u, c)][:, a * HD:(a + 1) * HD],
                                acc[:, c, 0:HD], rc[:, c, :],
                            )

            def tp_one(u, c):
                tch, j = u
                tpp = ps.tile([P, P], BF16, tag="aux", bufs=2,
                              name=f"tpp{tch}{j}{c}")
                nc.tensor.transpose(tpp[:], poTs.pop((u, c))[:], ident[:])
                nc.vector.tensor_copy(
                    po[:, j, tch * 512 + c * P: tch * 512 + (c + 1) * P],
                    tpp[:],
                )

            def tp_unit(u):
                for c in range(4):
                    tp_one(u, c)

            def fin_one(tch, c, tag="aux"):
                """Out-proj for one t-block: both 512-wide d-chunks, one store."""
                tsl = slice(tch * 512 + c * P, tch * 512 + (c + 1) * P)
                for dch in range(2):
                    fptag = tag if dch == 0 else ("uacc" if tch == 3 else tag)
                    fp = ps.tile([P, 512], F32, tag=fptag, bufs=2,
                                 name=f"fp{tch}{c}{dch}")
                    dsl = slice(dch * 512, (dch + 1) * 512)
                    for j in range(2):
                        nc.tensor.matmul(fp[:], po[:, j, tsl], wo[:, j, dsl],
                                         start=(j == 0), stop=(j == 1))
                    fo = sb.tile([P, 512], F32, tag="fo", bufs=4,
                                 name=f"fo{tch}{c}{dch}")
                    if tag == "uacc" and dch == 0:
                        nc.scalar.copy(fo[:], fp[:])
                    else:
                        nc.vector.tensor_copy(fo[:], fp[:])
                    nc.sync.dma_start(out_d[tsl, dsl], fo[:])

            # ================= PHASE A: QKV chunks + u00/u01/u10 scores ======
            u00, u01, u10, u11 = (0, 0), (0, 1), (1, 0), (1, 1)
            for u in (u00, u01, u10, u11):
                ebank[u] = {}
            alloc_accs(u00)

            # chunk 0
            qk_part(0, 0, 0)          # q0 m0 -> qT0
            qk_part(0, 1, 0)          # k0 m0 -> kT0
            dma_ft(0, 0)
            dma_ht(1)
            nc.sync.dma_start(bv[:], bv_d[:])
            nc.sync.dma_start(ident[:], id_d[:])
            for st in range(4):
                scores_step(u00, st)
            qk_cast0(0, 0)
            qk_resh(0, 0, 0)
            qk_cast0(1, 0)
            qk_resh(0, 1, 0)
            qk_part(0, 0, 1)
            qk_part(0, 1, 1)
            dma_ft(0, 1)
            for st in range(4):
                scores_step(u01, st)
            qk_cast0(0, 1)
            qk_resh(0, 0, 1)
            qk_cast0(1, 1)
            qk_resh(0, 1, 1)
            nc.sync.dma_start(wv[:], wvT_d.rearrange("(k p) r -> p k r", p=P))

            # chunks 1-3, serpentine: each chunk's k m-blocks are emitted
            # while the previous groups' exps are still queued, so the
            # bias+reshuffle chain hides; q(n>=1) and v are off-critical.
            qk_part(1, 1, 0)
            qk_resh(1, 1, 0)
            for n in range(1, 4):
                for st in range(4 * n, 4 * n + 4):
                    scores_step(u00, st)
                qk_part(n, 1, 1)
                qk_resh(n, 1, 1)
                qk_part(n, 0, 0)
                qk_part(n, 0, 1)
                qk_resh(n, 0)         # q merged, off critical path
                if n < 3:
                    dma_ht(n + 1)
                    dma_ft(0, n + 1)
                dma_ft(1, n - 1)
                for st in range(4 * n, 4 * n + 4):
                    scores_step(u01, st)
                for st in range(4 * n - 4, 4 * n):
                    scores_step(u10, st)
                for s in range(4 * n - 4, 4 * n):
                    v_stile(s)
                for st in range(4 * n - 4, 4 * n):
                    pv_step(u00, st)
                if n < 3:
                    qk_part(n + 1, 1, 0)
                    qk_resh(n + 1, 1, 0)
                if n == 3:
                    dma_ft(1, 3)

            # phase-A tail: last v group + u00 finish
            for s in range(12, 16):
                v_stile(s)
            for st in range(12, 16):
                pv_step(u00, st)
            nc.sync.dma_start(wo[:], woT_d.rearrange("(m p) d -> p m d", p=P))

            norm_unit(u00)
            tp_unit(u00)

            # ================= PHASE B: weave ===============================
            fresh = [u11, (2, 0), (2, 1), (3, 0), (3, 1)]
            work = [(u10, st) for st in range(12, 16)]
            for w in fresh:
                ebank.setdefault(w, {})
                work += [(w, st) for st in range(ST)]

            # replay queue: (unit, st) in replay order; fresh units appended
            # as their scores complete
            RORD = [0, 1, 2, 4, 5, 6, 7, 8, 10, 11, 12, 13, 3, 9, 14, 15]
            replayq = [(u01, st) for st in RORD]
            replayq += [(u10, st) for st in RORD]
            for w in fresh:
                replayq += [(w, st) for st in RORD]

            # fins become available per tch once both units' tps are done
            finq = []
            tp_done = {u00: True}
            fins_emitted = set()

            ft_sched = {
                (fresh[0], 2): (2, 0), (fresh[0], 6): (2, 1),
                (fresh[0], 10): (2, 2), (fresh[0], 14): (2, 3),
                ((2, 0), 2): (3, 0), ((2, 0), 6): (3, 1),
                ((2, 0), 10): (3, 2), ((2, 0), 14): (3, 3),
            }

            rpi = 0
            sci = 0
            cool = 0
            for (w, st) in work:
                scores_step(w, st)
                sci += 1
                # splice replay PV steps; a step can only replay once its e
                # tile is produced.  After a unit finishes (norm+transpose),
                # pause splicing so the next unit's first PV does not block
                # the PE pipeline while PSUM accumulators drain.
                budget = 2
                if cool > 0:
                    cool -= 1
                    budget = 0
                while budget > 0 and rpi < len(replayq):
                    ru, rst = replayq[rpi]
                    if rst not in ebank.get(ru, {}):
                        break  # not scored yet
                    if rst == 0:
                        alloc_accs(ru)
                    pv_step(ru, rst)
                    budget -= 1
                    rpi += 1
                    if rst == ST - 1:
                        norm_unit(ru)
                        tp_unit(ru)
                        tp_done[ru] = True
                        cool = 2
                        tch = ru[0]
                        other = (tch, 1 - ru[1])
                        if tp_done.get(other) and tch not in fins_emitted:
                            fins_emitted.add(tch)
                            finq += [(tch, c) for c in range(4)]
                        break
                # one fin (both d-chunks) every few scores steps
                if finq and sci % 4 == 0:
                    ftch, fc = finq.pop(0)
                    fin_one(ftch, fc, tag="uacc" if ftch == 3 else "aux")
                if (w, st) in ft_sched:
                    dma_ft(*ft_sched[(w, st)])

            # ---- tail: drain remaining replays, then c-pipelined tp+fin ----
            while rpi < len(replayq):
                ru, rst = replayq[rpi]
                if rst == 0:
                    alloc_accs(ru)
                pv_step(ru, rst)
                rpi += 1
                if rst == ST - 1:
                    norm_unit(ru, tail=True)
                    tch = ru[0]
                    other = (tch, 1 - ru[1])
                    if tp_done.get(other) and tch not in fins_emitted:
                        fins_emitted.add(tch)
                        tp_done[ru] = True
                        for c in range(4):
                            tp_one(ru, c)
                            fin_one(tch, c, tag="uacc")
                    else:
                        tp_unit(ru)
                        tp_done[ru] = True
                while finq:
                    ftch, fc = finq.pop(0)
                    fin_one(ftch, fc, tag="uacc" if ftch == 3 else "aux")
            while finq:
                ftch, fc = finq.pop(0)
                fin_one(ftch, fc, tag="uacc" if ftch == 3 else "aux")

    return nc


_NC = None
_LAST_RESULT = None


def _get_nc():
    global _NC
    if _NC is None:
        _NC = build_bass()
        if not _NC.is_finalized():
            _NC.finalize()
    return _NC


def kernel(hidden_states, focused_attention, Wq, bq, Wk, bk, Wv, bv, Wo, bo):
    bf = ml_dtypes.bfloat16
    hT = [np.ascontiguousarray(hidden_states[b].T).astype(bf) for b in range(B)]
    fT = [np.ascontiguousarray(focused_attention[b].T).astype(bf) for b in range(B)]

    in_maps = []
    for c in range(N_CORES):
        b, g = divmod(c, 4)
        rows = slice(g * R, (g + 1) * R)
        in_maps.append({
            "hT": hT[b],
            "fT": fT[b],
            "wqT": np.ascontiguousarray((Wq[rows] * (SCALING * QSC)).T).astype(bf),
            "wkT": np.ascontiguousarray((Wk[rows] * KSC).T).astype(bf),
            "wvT": np.ascontiguousarray(Wv[rows].T).astype(bf),
            "woT": np.ascontiguousarray(Wo[:, rows].T).astype(bf),
            "bq": np.ascontiguousarray(
                (bq[rows] * (SCALING * QSC))[:, None]).astype(np.float32),
            "bk": np.ascontiguousarray((bk[rows] * KSC)[:, None]).astype(np.float32),
            "bv": np.ascontiguousarray(bv[rows][None, :]).astype(bf),
            "ident": np.eye(P, dtype=bf),
        })

    res = run_bass_kernel_spmd(_get_nc(), in_maps, list(range(N_CORES)))
    global _LAST_RESULT
    _LAST_RESULT = res
    out = np.zeros((B, T, D), dtype=np.float32)
    for c in range(N_CORES):
        out[c // 4] += res.results[c]["out_partial"]
    out += np.asarray(bo, dtype=np.float32)[None, None, :]
    return out


# revision 4
# speedup vs baseline: 1.0035x; 1.0030x over previous
"""BartAttention (focused-attention variant) Trainium2 Bass kernel, v2.

Problem (hardcoded): B=2, T=2048, D=1024, H=16 heads, hd=64.
  q = (h @ Wq.T + bq) * hd**-0.5 ; k = h @ Wk.T + bk ; v = h @ Wv.T + bv
  scores = q @ k.T per head ; e = f * exp(scores) ; attn = e / rowsum(e)
  out = (attn @ v) @ Wo.T + bo

Sharding over 8 cores: batch (2) x head-group (4 groups of 4 heads); host
sums the 4 partial out-projections per batch and adds bo.

Per-core design (4 heads = 2 pairs j=0,1; ACT-exp is the bottleneck engine
at ~134us busy, everything else is scheduled to hide under it):
  - q/k projections bf16; q,k pre-scaled x16/x8 on the host and stored as
    fp8e4m3 in an hd-split [32 partitions, half, head, t] layout via
    SBUF->SBUF reshuffle DMAs
  - scores: fp8 DoubleRow matmuls (two 32-row contraction tiles per
    instruction -> 256 cycles per [128,512] block, 2x over bf16);
    sc = 128*q.k in PSUM f32; the first 4 s-tile groups of the (0,j) units
    use a bf16 path instead so the exp stream starts before any reshuffle
  - exp: ACT only, e = exp(sc/128) on [128,1024] tiles (2 heads);
    e *= fT in place on DVE (a few steps per unit go to GPSIMD to keep
    DVE under ACT)
  - PV e-stationary: acc[t-block, 65] += e_block.T @ [v|1]; N=65 matmuls;
    column 64 accumulates rowsum(e), so normalization is a per-partition
    reciprocal + tensor_scalar multiply (no broadcast matmuls)
  - po blocks transposed to [r, t] with PE transpose + DVE copy;
    out-proj per (t-block, d-chunk) accumulates both head pairs, DVE/ACT
    copies PSUM->SBUF, DMA out
  - schedule: QKV chunks serpentined with the scores of units (0,0), (0,1)
    and (1,0) (PV deferred into an SBUF e-backlog) so ACT never starves in
    phase A; phase B replays each unit's PV one unit behind the fresh
    scores stream, with out-projections and transposes woven between.
"""

import numpy as np
import ml_dtypes

import concourse.bass as bass
import concourse.bacc as bacc
import concourse.mybir as mybir
from concourse.tile import TileContext
from concourse.bass_utils import run_bass_kernel_spmd

BF16 = mybir.dt.bfloat16
F32 = mybir.dt.float32
F8 = mybir.dt.float8e4
AF = mybir.ActivationFunctionType
DR = mybir.MatmulPerfMode.DoubleRow

B, T, D = 2, 2048, 1024
H, HD = 16, 64
HG = 4               # heads per core
R = HG * HD          # 256 rows per core
SCALING = HD ** -0.5
N_CORES = 8

P = 128
KT = D // P          # 8 k-tiles for QKV contraction
NCH = T // 512       # 4 t-chunks
ST = T // P          # 16 s-tiles

QSC, KSC = 16.0, 8.0         # fp8 pre-scales for q and k
EXP_SCALE = 1.0 / (QSC * KSC)


def build_bass():
    nc = bacc.Bacc()

    hT_d = nc.declare_dram_parameter("hT", [D, T], BF16, isOutput=False)
    fT_d = nc.declare_dram_parameter("fT", [T, T], BF16, isOutput=False)
    wqT_d = nc.declare_dram_parameter("wqT", [D, R], BF16, isOutput=False)
    wkT_d = nc.declare_dram_parameter("wkT", [D, R], BF16, isOutput=False)
    wvT_d = nc.declare_dram_parameter("wvT", [D, R], BF16, isOutput=False)
    woT_d = nc.declare_dram_parameter("woT", [R, D], BF16, isOutput=False)
    bq_d = nc.declare_dram_parameter("bq", [R, 1], F32, isOutput=False)
    bk_d = nc.declare_dram_parameter("bk", [R, 1], F32, isOutput=False)
    bv_d = nc.declare_dram_parameter("bv", [1, R], BF16, isOutput=False)
    id_d = nc.declare_dram_parameter("ident", [P, P], BF16, isOutput=False)
    out_d = nc.declare_dram_parameter("out_partial", [T, D], F32, isOutput=True)

    hT_r = hT_d.rearrange("(k p) t -> p k t", p=P)
    fT_r = fT_d.rearrange("(s p) t -> p s t", p=P)

    with TileContext(nc) as tc:
        with (
            nc.allow_low_precision(reason="bf16/fp8 pipeline is intentional"),
            tc.tile_pool(name="sb", bufs=1) as sb,
            tc.tile_pool(name="ps", bufs=1, space="PSUM") as ps,
        ):
            # ---- persistent SBUF ----
            wq = sb.tile([P, KT, R], BF16)
            wk = sb.tile([P, KT, R], BF16)
            wv = sb.tile([P, KT, R], BF16)
            wo = sb.tile([P, 2, D], BF16)
            bq = sb.tile([P, 2], F32)
            bk = sb.tile([P, 2], F32)
            bv = sb.tile([1, R], BF16)
            ident = sb.tile([P, P], BF16)
            ones_r = sb.tile([1, P], BF16)
            q8s = sb.tile([32, 2, HG, T], F8)   # [p32, half, head, t]
            k8s = sb.tile([32, 2, HG, T], F8)
            vsb = sb.tile([P, ST, HG, HD + 1], BF16)
            qT0 = sb.tile([P, 2, 512], BF16)    # chunk-0 q bf16 (ramp path)
            kT0 = sb.tile([P, 2, 512], BF16)
            po = sb.tile([P, 2, T], BF16)       # out-proj lhsT [r, (tch c t)]

            nc.vector.memset(ones_r[:], 1.0)
            nc.vector.memset(vsb[:, :, :, HD:HD + 1], 1.0)

            # PE warm-up: burn the p-state ramp on junk matmuls during DMA
            for i in range(24):
                wrm = ps.tile([P, P], F32, tag="sc", bufs=2, name=f"warm{i}")
                nc.tensor.matmul(wrm[:], ones_r[:], ones_r[:],
                                 start=True, stop=True)

            # ---- initial DMAs ----
            nc.sync.dma_start(wq[:], wqT_d.rearrange("(k p) r -> p k r", p=P))
            ht = {}

            def dma_ht(n):
                t = sb.tile([P, KT, 512], BF16, tag="ht", bufs=3, name=f"ht{n}")
                nsl = slice(n * 512, (n + 1) * 512)
                nc.sync.dma_start(t[:, 0:4, :], hT_r[:, 0:4, nsl])
                nc.sync.dma_start(t[:, 4:8, :], hT_r[:, 4:8, nsl])
                ht[n] = t

            ht0 = sb.tile([P, KT, 512], BF16, tag="ht", bufs=3, name="ht0")
            nc.sync.dma_start(ht0[:, 0:4, :], hT_r[:, 0:4, 0:512])
            nc.sync.dma_start(wk[:], wkT_d.rearrange("(k p) r -> p k r", p=P))
            nc.sync.dma_start(ht0[:, 4:8, :], hT_r[:, 4:8, 0:512])
            ht[0] = ht0
            nc.sync.dma_start(bq[:], bq_d.rearrange("(m p) one -> p (m one)", p=P))
            nc.sync.dma_start(bk[:], bk_d.rearrange("(m p) one -> p (m one)", p=P))

            fts = {}

            def dma_ft(tch, g):
                t = sb.tile([P, 4, 512], BF16, tag="ft", bufs=8,
                            name=f"ft{tch}g{g}")
                nc.sync.dma_start(
                    t[:], fT_r[:, 4 * g:4 * g + 4, tch * 512:(tch + 1) * 512]
                )
                fts[(tch, g)] = t

            # ---- helpers ----
            st8s = {}

            def qk_part(n, tens, m, cols=None):
                """q or k projection matmuls + bias for chunk n, m-block.
                cols: optional (lo, hi) sub-range of the 512 chunk columns."""
                w_sb, b_sb = (wq, bq) if tens == 0 else (wk, bk)
                lo, hi = cols if cols else (0, 512)
                nm = f"{'qk'[tens]}{n}m{m}c{lo}"
                acc = ps.tile([P, hi - lo], F32, tag="aux", bufs=2,
                              name=f"a{nm}")
                for kk in range(KT):
                    nc.tensor.matmul(
                        acc[:], w_sb[:, kk, m * P:(m + 1) * P],
                        ht[n][:, kk, lo:hi],
                        start=(kk == 0), stop=(kk == KT - 1),
                    )
                if n == 0:
                    dst = qT0 if tens == 0 else kT0
                    nc.vector.tensor_scalar_add(dst[:, m, lo:hi], acc[:],
                                                b_sb[:, m:m + 1])
                    return
                if (tens, n) not in st8s:
                    st8s[(tens, n)] = sb.tile([P, 2, 512], F8, tag="st8",
                                              bufs=4, name=f"s{'qk'[tens]}{n}")
                nc.vector.tensor_scalar_add(st8s[(tens, n)][:, m, lo:hi],
                                            acc[:], b_sb[:, m:m + 1])

            def qk_cast0(tens, m):
                if (tens, 0) not in st8s:
                    st8s[(tens, 0)] = sb.tile([P, 2, 512], F8, tag="st8",
                                              bufs=4, name=f"s{'qk'[tens]}0")
                src_t = qT0 if tens == 0 else kT0
                nc.vector.tensor_copy(st8s[(tens, 0)][:, m, :], src_t[:, m, :])

            def qk_resh(n, tens, m=None, cols=None):
                """Reshuffle chunk n into the [32, half, head, t] hd-split
                layout. m=None: both m-blocks in 4 DMAs of [32, 2, 512]
                (m via stride-2 head dim); m=int: that m-block only."""
                dst = q8s if tens == 0 else k8s
                st8 = st8s[(tens, n)]
                lo, hi = cols if cols else (0, 512)
                nsl = slice(n * 512 + lo, n * 512 + hi)
                for half in range(2):
                    for hm in range(2):
                        src_p = slice(64 * hm + 32 * half, 64 * hm + 32 * half + 32)
                        if m is None:
                            nc.sync.dma_start(dst[:, half, hm::2, nsl],
                                              st8[src_p, :, lo:hi])
                        else:
                            nc.sync.dma_start(dst[:, half, 2 * m + hm, nsl],
                                              st8[src_p, m, lo:hi])

            def v_stile(s):
                acc = ps.tile([P, R], F32, tag="aux", bufs=2, name=f"vacc{s}")
                for kk in range(KT):
                    nc.tensor.matmul(
                        acc[:], ht[s // 4][:, kk, (s % 4) * P:(s % 4 + 1) * P],
                        wv[:, kk, :], start=(kk == 0), stop=False,
                    )
                nc.tensor.matmul(acc[:], ones_r[:], bv[:], start=False, stop=True)
                nc.vector.tensor_copy(
                    vsb[:, s, :, 0:HD],
                    acc[:].rearrange("p (h d) -> p h d", h=HG),
                )

            ebank = {}    # u -> {st: e_tile} pending PV
            accs = {}     # u -> (acc_a, acc_b)
            poTs = {}     # (u, c) -> poT tile

            def scores_step(u, st):
                tch, j = u
                sc = ps.tile([P, 1024], F32, tag="sc", bufs=2,
                             name=f"sc{tch}{j}_{st}")
                tsl = slice(tch * 512, (tch + 1) * 512)
                ssl = slice(st * P, (st + 1) * P)
                for a in range(2):
                    h = 2 * j + a
                    if tch == 0 and st < 4:
                        rows = slice(a * HD, (a + 1) * HD)
                        nc.tensor.matmul(
                            sc[:, a * 512:(a + 1) * 512],
                            kT0[rows, j, st * P:(st + 1) * P],
                            qT0[rows, j, :],
                            start=True, stop=True,
                        )
                    else:
                        nc.tensor.matmul(
                            sc[:, a * 512:(a + 1) * 512],
                            k8s[:, :, h, ssl], q8s[:, :, h, tsl],
                            start=True, stop=True, perf_mode=DR,
                        )
                e = sb.tile([P, 1024], BF16, tag="e", bufs=32,
                            name=f"e{tch}{j}_{st}")
                nc.scalar.activation(e[:], sc[:], AF.Exp, scale=EXP_SCALE)
                ftt = fts[(tch, st // 4)]
                eng = nc.gpsimd if st in (3, 9, 14) else nc.vector
                for a in range(2):
                    half = slice(a * 512, (a + 1) * 512)
                    eng.tensor_mul(e[:, half], e[:, half],
                                   ftt[:, st % 4, :])
                ebank[u][st] = e

            def alloc_accs(u):
                accs[u] = tuple(
                    ps.tile([P, 4, HD + 1], F32, tag="uacc", bufs=2,
                            name=f"acc{u[0]}{u[1]}{a}")
                    for a in range(2)
                )

            def pv_step(u, st, first=None, last=None):
                tch, j = u
                first = (st == 0) if first is None else first
                last = (st == ST - 1) if last is None else last
                e = ebank[u].pop(st)
                for a in range(2):
                    acc = accs[u][a]
                    for c in range(4):
                        nc.tensor.matmul(
                            acc[:, c, :],
                            e[:, a * 512 + c * P:a * 512 + (c + 1) * P],
                            vsb[:, st, 2 * j + a, :],
                            start=(first and c == 0),
                            stop=(last and c == 3),
                            skip_group_check=True,
                        )

            def norm_unit(u, tail=False):
                for a in range(2):
                    acc = accs[u][a]
                    rc = sb.tile([P, 4, 1], F32, tag="rc", bufs=4,
                                 name=f"rc{u[0]}{u[1]}{a}")
                    nc.vector.reciprocal(rc[:], acc[:, :, HD:HD + 1])
                    for c in range(4):
                        if (u, c) not in poTs:
                            poTs[(u, c)] = sb.tile(
                                [P, P], BF16, tag="pt", bufs=8,
                                name=f"pt{u[0]}{u[1]}{c}")
                        if tail and a == 1:
                            nc.scalar.mul(
                                poTs[(u, c)][:, a * HD:(a + 1) * HD],
                                acc[:, c, 0:HD], rc[:, c, :],
                            )
                        else:
                            nc.vector.tensor_scalar_mul(
                                poTs[(u, c)][:, a * HD:(a + 1) * HD],
                                acc[:, c, 0:HD], rc[:, c, :],
                            )

            def tp_one(u, c):
                tch, j = u
                tpp = ps.tile([P, P], BF16, tag="aux", bufs=2,
                              name=f"tpp{tch}{j}{c}")
                nc.tensor.transpose(tpp[:], poTs.pop((u, c))[:], ident[:])
                nc.vector.tensor_copy(
                    po[:, j, tch * 512 + c * P: tch * 512 + (c + 1) * P],
                    tpp[:],
                )

            def tp_unit(u):
                for c in range(4):
                    tp_one(u, c)

            def fin_one(tch, c, tag="aux"):
                """Out-proj for one t-block: both 512-wide d-chunks, one store."""
                tsl = slice(tch * 512 + c * P, tch * 512 + (c + 1) * P)
                for dch in range(2):
                    fptag = tag if dch == 0 else ("uacc" if tch == 3 else tag)
                    fp = ps.tile([P, 512], F32, tag=fptag, bufs=2,
                                 name=f"fp{tch}{c}{dch}")
                    dsl = slice(dch * 512, (dch + 1) * 512)
                    for j in range(2):
                        nc.tensor.matmul(fp[:], po[:, j, tsl], wo[:, j, dsl],
                                         start=(j == 0), stop=(j == 1))
                    fo = sb.tile([P, 512], F32, tag="fo", bufs=4,
                                 name=f"fo{tch}{c}{dch}")
                    if tag == "uacc" and dch == 0:
                        nc.scalar.copy(fo[:], fp[:])
                    else:
                        nc.vector.tensor_copy(fo[:], fp[:])
                    nc.sync.dma_start(out_d[tsl, dsl], fo[:])

            # ================= PHASE A: QKV chunks + u00/u01/u10 scores ======
            u00, u01, u10, u11 = (0, 0), (0, 1), (1, 0), (1, 1)
            for u in (u00, u01, u10, u11):
                ebank[u] = {}
            alloc_accs(u00)

            # chunk 0
            qk_part(0, 0, 0)          # q0 m0 -> qT0
            qk_part(0, 1, 0)          # k0 m0 -> kT0
            dma_ft(0, 0)
            dma_ht(1)
            nc.sync.dma_start(bv[:], bv_d[:])
            nc.sync.dma_start(ident[:], id_d[:])
            for st in range(4):
                scores_step(u00, st)
            qk_cast0(0, 0)
            qk_resh(0, 0, 0)
            qk_cast0(1, 0)
            qk_resh(0, 1, 0)
            qk_part(0, 0, 1)
            qk_part(0, 1, 1)
            dma_ft(0, 1)
            for st in range(4):
                scores_step(u01, st)
            qk_cast0(0, 1)
            qk_resh(0, 0, 1)
            qk_cast0(1, 1)
            qk_resh(0, 1, 1)
            # chunks 1-3, serpentine: each chunk's k m-blocks are emitted
            # while the previous groups' exps are still queued, so the
            # bias+reshuffle chain hides; q(n>=1) and v are off-critical.
            qk_part(1, 1, 0)
            qk_resh(1, 1, 0)
            nc.sync.dma_start(wv[:], wvT_d.rearrange("(k p) r -> p k r", p=P))
            for n in range(1, 4):
                for st in range(4 * n, 4 * n + 4):
                    scores_step(u00, st)
                qk_part(n, 1, 1)
                qk_resh(n, 1, 1)
                qk_part(n, 0, 0)
                qk_part(n, 0, 1)
                qk_resh(n, 0)         # q merged, off critical path
                if n < 3:
                    dma_ht(n + 1)
                    dma_ft(0, n + 1)
                dma_ft(1, n - 1)
                for st in range(4 * n, 4 * n + 4):
                    scores_step(u01, st)
                for st in range(4 * n - 4, 4 * n):
                    scores_step(u10, st)
                for s in range(4 * n - 4, 4 * n):
                    v_stile(s)
                for st in range(4 * n - 4, 4 * n):
                    pv_step(u00, st)
                if n < 3:
                    qk_part(n + 1, 1, 0)
                    qk_resh(n + 1, 1, 0)
                if n == 3:
                    dma_ft(1, 3)

            # phase-A tail: last v group + u00 finish
            for s in range(12, 16):
                v_stile(s)
            for st in range(12, 16):
                pv_step(u00, st)
            nc.sync.dma_start(wo[:], woT_d.rearrange("(m p) d -> p m d", p=P))

            norm_unit(u00)
            tp_unit(u00)

            # ================= PHASE B: weave ===============================
            fresh = [u11, (2, 0), (2, 1), (3, 0), (3, 1)]
            work = [(u10, st) for st in range(12, 16)]
            for w in fresh:
                ebank.setdefault(w, {})
                work += [(w, st) for st in range(ST)]

            # replay queue: (unit, st) in replay order; fresh units appended
            # as their scores complete
            RORD = [0, 1, 2, 4, 5, 6, 7, 8, 10, 11, 12, 13, 3, 9, 14, 15]
            replayq = [(u01, st) for st in RORD]
            replayq += [(u10, st) for st in RORD]
            for w in fresh:
                replayq += [(w, st) for st in RORD]

            # fins become available per tch once both units' tps are done
            finq = []
            tp_done = {u00: True}
            fins_emitted = set()

            ft_sched = {
                (fresh[0], 2): (2, 0), (fresh[0], 6): (2, 1),
                (fresh[0], 10): (2, 2), (fresh[0], 14): (2, 3),
                ((2, 0), 2): (3, 0), ((2, 0), 6): (3, 1),
                ((2, 0), 10): (3, 2), ((2, 0), 14): (3, 3),
            }

            rpi = 0
            sci = 0
            cool = 0
            for (w, st) in work:
                scores_step(w, st)
                sci += 1
                # splice replay PV steps; a step can only replay once its e
                # tile is produced.  After a unit finishes (norm+transpose),
                # pause splicing so the next unit's first PV does not block
                # the PE pipeline while PSUM accumulators drain.
                budget = 2
                if cool > 0:
                    cool -= 1
                    budget = 0
                while budget > 0 and rpi < len(replayq):
                    ru, rst = replayq[rpi]
                    if rst not in ebank.get(ru, {}):
                        break  # not scored yet
                    if rst == 0:
                        alloc_accs(ru)
                    pv_step(ru, rst)
                    budget -= 1
                    rpi += 1
                    if rst == ST - 1:
                        norm_unit(ru)
                        tp_unit(ru)
                        tp_done[ru] = True
                        cool = 2
                        tch = ru[0]
                        other = (tch, 1 - ru[1])
                        if tp_done.get(other) and tch not in fins_emitted:
                            fins_emitted.add(tch)
                            finq += [(tch, c) for c in range(4)]
                        break
                # one fin (both d-chunks) every few scores steps
                if finq and sci % 4 == 0:
                    ftch, fc = finq.pop(0)
                    fin_one(ftch, fc, tag="uacc" if ftch == 3 else "aux")
                if (w, st) in ft_sched:
                    dma_ft(*ft_sched[(w, st)])

            # ---- tail: drain remaining replays, then c-pipelined tp+fin ----
            while rpi < len(replayq):
                ru, rst = replayq[rpi]
                if rst == 0:
                    alloc_accs(ru)
                pv_step(ru, rst)
                rpi += 1
                if rst == ST - 1:
                    norm_unit(ru, tail=True)
                    tch = ru[0]
                    other = (tch, 1 - ru[1])
                    if tp_done.get(other) and tch not in fins_emitted:
                        fins_emitted.add(tch)
                        tp_done[ru] = True
                        for c in range(4):
                            tp_one(ru, c)
                            fin_one(tch, c, tag="uacc")
                    else:
                        tp_unit(ru)
                        tp_done[ru] = True
                while finq:
                    ftch, fc = finq.pop(0)
                    fin_one(ftch, fc, tag="uacc" if ftch == 3 else "aux")
            while finq:
                ftch, fc = finq.pop(0)
                fin_one(ftch, fc, tag="uacc" if ftch == 3 else "aux")

    return nc


_NC = None
_LAST_RESULT = None


def _get_nc():
    global _NC
    if _NC is None:
        _NC = build_bass()
        if not _NC.is_finalized():
            _NC.finalize()
    return _NC


def kernel(hidden_states, focused_attention, Wq, bq, Wk, bk, Wv, bv, Wo, bo):
    bf = ml_dtypes.bfloat16
    hT = [np.ascontiguousarray(hidden_states[b].T).astype(bf) for b in range(B)]
    fT = [np.ascontiguousarray(focused_attention[b].T).astype(bf) for b in range(B)]

    in_maps = []
    for c in range(N_CORES):
        b, g = divmod(c, 4)
        rows = slice(g * R, (g + 1) * R)
        in_maps.append({
            "hT": hT[b],
            "fT": fT[b],
            "wqT": np.ascontiguousarray((Wq[rows] * (SCALING * QSC)).T).astype(bf),
            "wkT": np.ascontiguousarray((Wk[rows] * KSC).T).astype(bf),
            "wvT": np.ascontiguousarray(Wv[rows].T).astype(bf),
            "woT": np.ascontiguousarray(Wo[:, rows].T).astype(bf),
            "bq": np.ascontiguousarray(
                (bq[rows] * (SCALING * QSC))[:, None]).astype(np.float32),
            "bk": np.ascontiguousarray((bk[rows] * KSC)[:, None]).astype(np.float32),
            "bv": np.ascontiguousarray(bv[rows][None, :]).astype(bf),
            "ident": np.eye(P, dtype=bf),
        })

    res = run_bass_kernel_spmd(_get_nc(), in_maps, list(range(N_CORES)))
    global _LAST_RESULT
    _LAST_RESULT = res
    out = np.zeros((B, T, D), dtype=np.float32)
    for c in range(N_CORES):
        out[c // 4] += res.results[c]["out_partial"]
    out += np.asarray(bo, dtype=np.float32)[None, None, :]
    return out


# revision 5
# speedup vs baseline: 1.0040x; 1.0006x over previous
"""BartAttention (focused-attention variant) Trainium2 Bass kernel, v2.

Problem (hardcoded): B=2, T=2048, D=1024, H=16 heads, hd=64.
  q = (h @ Wq.T + bq) * hd**-0.5 ; k = h @ Wk.T + bk ; v = h @ Wv.T + bv
  scores = q @ k.T per head ; e = f * exp(scores) ; attn = e / rowsum(e)
  out = (attn @ v) @ Wo.T + bo

Sharding over 8 cores: batch (2) x head-group (4 groups of 4 heads); host
sums the 4 partial out-projections per batch and adds bo.

Per-core design (4 heads = 2 pairs j=0,1; ACT-exp is the bottleneck engine
at ~134us busy, everything else is scheduled to hide under it):
  - q/k projections bf16; q,k pre-scaled x16/x8 on the host and stored as
    fp8e4m3 in an hd-split [32 partitions, half, head, t] layout via
    SBUF->SBUF reshuffle DMAs
  - scores: fp8 DoubleRow matmuls (two 32-row contraction tiles per
    instruction -> 256 cycles per [128,512] block, 2x over bf16);
    sc = 128*q.k in PSUM f32; the first 4 s-tile groups of the (0,j) units
    use a bf16 path instead so the exp stream starts before any reshuffle
  - exp: ACT only, e = exp(sc/128) on [128,1024] tiles (2 heads);
    e *= fT in place on DVE (a few steps per unit go to GPSIMD to keep
    DVE under ACT)
  - PV e-stationary: acc[t-block, 65] += e_block.T @ [v|1]; N=65 matmuls;
    column 64 accumulates rowsum(e), so normalization is a per-partition
    reciprocal + tensor_scalar multiply (no broadcast matmuls)
  - po blocks transposed to [r, t] with PE transpose + DVE copy;
    out-proj per (t-block, d-chunk) accumulates both head pairs, DVE/ACT
    copies PSUM->SBUF, DMA out
  - schedule: QKV chunks serpentined with the scores of units (0,0), (0,1)
    and (1,0) (PV deferred into an SBUF e-backlog) so ACT never starves in
    phase A; phase B replays each unit's PV one unit behind the fresh
    scores stream, with out-projections and transposes woven between.
"""

import numpy as np
import ml_dtypes

import concourse.bass as bass
import concourse.bacc as bacc
import concourse.mybir as mybir
from concourse.tile import TileContext
from concourse.bass_utils import run_bass_kernel_spmd

BF16 = mybir.dt.bfloat16
F32 = mybir.dt.float32
F8 = mybir.dt.float8e4
AF = mybir.ActivationFunctionType
DR = mybir.MatmulPerfMode.DoubleRow

B, T, D = 2, 2048, 1024
H, HD = 16, 64
HG = 4               # heads per core
R = HG * HD          # 256 rows per core
SCALING = HD ** -0.5
N_CORES = 8

P = 128
KT = D // P          # 8 k-tiles for QKV contraction
NCH = T // 512       # 4 t-chunks
ST = T // P          # 16 s-tiles

QSC, KSC = 16.0, 8.0         # fp8 pre-scales for q and k
EXP_SCALE = 1.0 / (QSC * KSC)


def build_bass():
    nc = bacc.Bacc()

    hT_d = nc.declare_dram_parameter("hT", [D, T], BF16, isOutput=False)
    fT_d = nc.declare_dram_parameter("fT", [T, T], BF16, isOutput=False)
    wqT_d = nc.declare_dram_parameter("wqT", [D, R], BF16, isOutput=False)
    wkT_d = nc.declare_dram_parameter("wkT", [D, R], BF16, isOutput=False)
    wvT_d = nc.declare_dram_parameter("wvT", [D, R], BF16, isOutput=False)
    woT_d = nc.declare_dram_parameter("woT", [R, D], BF16, isOutput=False)
    bq_d = nc.declare_dram_parameter("bq", [R, 1], F32, isOutput=False)
    bk_d = nc.declare_dram_parameter("bk", [R, 1], F32, isOutput=False)
    bv_d = nc.declare_dram_parameter("bv", [1, R], BF16, isOutput=False)
    id_d = nc.declare_dram_parameter("ident", [P, P], BF16, isOutput=False)
    out_d = nc.declare_dram_parameter("out_partial", [T, D], F32, isOutput=True)

    hT_r = hT_d.rearrange("(k p) t -> p k t", p=P)
    fT_r = fT_d.rearrange("(s p) t -> p s t", p=P)

    with TileContext(nc) as tc:
        with (
            nc.allow_low_precision(reason="bf16/fp8 pipeline is intentional"),
            tc.tile_pool(name="sb", bufs=1) as sb,
            tc.tile_pool(name="ps", bufs=1, space="PSUM") as ps,
        ):
            # ---- persistent SBUF ----
            wq = sb.tile([P, KT, R], BF16)
            wk = sb.tile([P, KT, R], BF16)
            wv = sb.tile([P, KT, R], BF16)
            wo = sb.tile([P, 2, D], BF16)
            bq = sb.tile([P, 2], F32)
            bk = sb.tile([P, 2], F32)
            bv = sb.tile([1, R], BF16)
            ident = sb.tile([P, P], BF16)
            ones_r = sb.tile([1, P], BF16)
            q8s = sb.tile([32, 2, HG, T], F8)   # [p32, half, head, t]
            k8s = sb.tile([32, 2, HG, T], F8)
            vsb = sb.tile([P, ST, HG, HD + 1], BF16)
            qT0 = sb.tile([P, 2, 512], BF16)    # chunk-0 q bf16 (ramp path)
            kT0 = sb.tile([P, 2, 1024], BF16)   # chunks 0-1 k bf16 (seam path)
            po = sb.tile([P, 2, T], BF16)       # out-proj lhsT [r, (tch c t)]

            nc.vector.memset(ones_r[:], 1.0)
            nc.vector.memset(vsb[:, :, :, HD:HD + 1], 1.0)

            # PE warm-up: burn the p-state ramp on junk matmuls during DMA
            for i in range(24):
                wrm = ps.tile([P, P], F32, tag="sc", bufs=2, name=f"warm{i}")
                nc.tensor.matmul(wrm[:], ones_r[:], ones_r[:],
                                 start=True, stop=True)

            # ---- initial DMAs ----
            nc.sync.dma_start(wq[:], wqT_d.rearrange("(k p) r -> p k r", p=P))
            ht = {}

            def dma_ht(n):
                t = sb.tile([P, KT, 512], BF16, tag="ht", bufs=3, name=f"ht{n}")
                nsl = slice(n * 512, (n + 1) * 512)
                nc.sync.dma_start(t[:, 0:4, :], hT_r[:, 0:4, nsl])
                nc.sync.dma_start(t[:, 4:8, :], hT_r[:, 4:8, nsl])
                ht[n] = t

            ht0 = sb.tile([P, KT, 512], BF16, tag="ht", bufs=3, name="ht0")
            nc.sync.dma_start(ht0[:, 0:4, :], hT_r[:, 0:4, 0:512])
            nc.sync.dma_start(wk[:], wkT_d.rearrange("(k p) r -> p k r", p=P))
            nc.sync.dma_start(ht0[:, 4:8, :], hT_r[:, 4:8, 0:512])
            ht[0] = ht0
            nc.sync.dma_start(bq[:], bq_d.rearrange("(m p) one -> p (m one)", p=P))
            nc.sync.dma_start(bk[:], bk_d.rearrange("(m p) one -> p (m one)", p=P))

            fts = {}

            def dma_ft(tch, g):
                t = sb.tile([P, 4, 512], BF16, tag="ft", bufs=8,
                            name=f"ft{tch}g{g}")
                nc.sync.dma_start(
                    t[:], fT_r[:, 4 * g:4 * g + 4, tch * 512:(tch + 1) * 512]
                )
                fts[(tch, g)] = t

            # ---- helpers ----
            st8s = {}

            def qk_part(n, tens, m, cols=None):
                """q or k projection matmuls + bias for chunk n, m-block.
                cols: optional (lo, hi) sub-range of the 512 chunk columns."""
                w_sb, b_sb = (wq, bq) if tens == 0 else (wk, bk)
                lo, hi = cols if cols else (0, 512)
                nm = f"{'qk'[tens]}{n}m{m}c{lo}"
                acc = ps.tile([P, hi - lo], F32, tag="aux", bufs=2,
                              name=f"a{nm}")
                for kk in range(KT):
                    nc.tensor.matmul(
                        acc[:], w_sb[:, kk, m * P:(m + 1) * P],
                        ht[n][:, kk, lo:hi],
                        start=(kk == 0), stop=(kk == KT - 1),
                    )
                if n == 0 or (n == 1 and tens == 1):
                    dst = qT0 if tens == 0 else kT0
                    off = 0 if n == 0 else 512
                    nc.vector.tensor_scalar_add(dst[:, m, off + lo:off + hi],
                                                acc[:], b_sb[:, m:m + 1])
                    return
                if (tens, n) not in st8s:
                    st8s[(tens, n)] = sb.tile([P, 2, 512], F8, tag="st8",
                                              bufs=4, name=f"s{'qk'[tens]}{n}")
                nc.vector.tensor_scalar_add(st8s[(tens, n)][:, m, lo:hi],
                                            acc[:], b_sb[:, m:m + 1])

            def qk_cast0(tens, m, n=0):
                if (tens, n) not in st8s:
                    st8s[(tens, n)] = sb.tile([P, 2, 512], F8, tag="st8",
                                              bufs=4, name=f"s{'qk'[tens]}{n}")
                src_t = qT0 if tens == 0 else kT0
                off = 0 if n == 0 else 512
                nc.vector.tensor_copy(st8s[(tens, n)][:, m, :],
                                      src_t[:, m, off:off + 512])

            def qk_resh(n, tens, m=None, cols=None):
                """Reshuffle chunk n into the [32, half, head, t] hd-split
                layout. m=None: both m-blocks in 4 DMAs of [32, 2, 512]
                (m via stride-2 head dim); m=int: that m-block only."""
                dst = q8s if tens == 0 else k8s
                st8 = st8s[(tens, n)]
                lo, hi = cols if cols else (0, 512)
                nsl = slice(n * 512 + lo, n * 512 + hi)
                for half in range(2):
                    for hm in range(2):
                        src_p = slice(64 * hm + 32 * half, 64 * hm + 32 * half + 32)
                        if m is None:
                            nc.sync.dma_start(dst[:, half, hm::2, nsl],
                                              st8[src_p, :, lo:hi])
                        else:
                            nc.sync.dma_start(dst[:, half, 2 * m + hm, nsl],
                                              st8[src_p, m, lo:hi])

            def v_stile(s):
                acc = ps.tile([P, R], F32, tag="aux", bufs=2, name=f"vacc{s}")
                for kk in range(KT):
                    nc.tensor.matmul(
                        acc[:], ht[s // 4][:, kk, (s % 4) * P:(s % 4 + 1) * P],
                        wv[:, kk, :], start=(kk == 0), stop=False,
                    )
                nc.tensor.matmul(acc[:], ones_r[:], bv[:], start=False, stop=True)
                nc.vector.tensor_copy(
                    vsb[:, s, :, 0:HD],
                    acc[:].rearrange("p (h d) -> p h d", h=HG),
                )

            ebank = {}    # u -> {st: e_tile} pending PV
            accs = {}     # u -> (acc_a, acc_b)
            poTs = {}     # (u, c) -> poT tile

            def scores_step(u, st):
                tch, j = u
                sc = ps.tile([P, 1024], F32, tag="sc", bufs=2,
                             name=f"sc{tch}{j}_{st}")
                tsl = slice(tch * 512, (tch + 1) * 512)
                ssl = slice(st * P, (st + 1) * P)
                for a in range(2):
                    h = 2 * j + a
                    if tch == 0 and st < 8:
                        rows = slice(a * HD, (a + 1) * HD)
                        nc.tensor.matmul(
                            sc[:, a * 512:(a + 1) * 512],
                            kT0[rows, j, st * P:(st + 1) * P],
                            qT0[rows, j, :],
                            start=True, stop=True,
                        )
                    else:
                        nc.tensor.matmul(
                            sc[:, a * 512:(a + 1) * 512],
                            k8s[:, :, h, ssl], q8s[:, :, h, tsl],
                            start=True, stop=True, perf_mode=DR,
                        )
                e = sb.tile([P, 1024], BF16, tag="e", bufs=32,
                            name=f"e{tch}{j}_{st}")
                nc.scalar.activation(e[:], sc[:], AF.Exp, scale=EXP_SCALE)
                ftt = fts[(tch, st // 4)]
                eng = nc.gpsimd if st in (3, 9, 14) else nc.vector
                for a in range(2):
                    half = slice(a * 512, (a + 1) * 512)
                    eng.tensor_mul(e[:, half], e[:, half],
                                   ftt[:, st % 4, :])
                ebank[u][st] = e

            def alloc_accs(u):
                accs[u] = tuple(
                    ps.tile([P, 4, HD + 1], F32, tag="uacc", bufs=2,
                            name=f"acc{u[0]}{u[1]}{a}")
                    for a in range(2)
                )

            def pv_step(u, st, first=None, last=None):
                tch, j = u
                first = (st == 0) if first is None else first
                last = (st == ST - 1) if last is None else last
                e = ebank[u].pop(st)
                for a in range(2):
                    acc = accs[u][a]
                    for c in range(4):
                        nc.tensor.matmul(
                            acc[:, c, :],
                            e[:, a * 512 + c * P:a * 512 + (c + 1) * P],
                            vsb[:, st, 2 * j + a, :],
                            start=(first and c == 0),
                            stop=(last and c == 3),
                            skip_group_check=True,
                        )

            def norm_unit(u, tail=False):
                for a in range(2):
                    acc = accs[u][a]
                    rc = sb.tile([P, 4, 1], F32, tag="rc", bufs=4,
                                 name=f"rc{u[0]}{u[1]}{a}")
                    nc.vector.reciprocal(rc[:], acc[:, :, HD:HD + 1])
                    for c in range(4):
                        if (u, c) not in poTs:
                            poTs[(u, c)] = sb.tile(
                                [P, P], BF16, tag="pt", bufs=8,
                                name=f"pt{u[0]}{u[1]}{c}")
                        if tail and a == 1:
                            nc.scalar.mul(
                                poTs[(u, c)][:, a * HD:(a + 1) * HD],
                                acc[:, c, 0:HD], rc[:, c, :],
                            )
                        else:
                            nc.vector.tensor_scalar_mul(
                                poTs[(u, c)][:, a * HD:(a + 1) * HD],
                                acc[:, c, 0:HD], rc[:, c, :],
                            )

            def tp_one(u, c):
                tch, j = u
                tpp = ps.tile([P, P], BF16, tag="aux", bufs=2,
                              name=f"tpp{tch}{j}{c}")
                nc.tensor.transpose(tpp[:], poTs.pop((u, c))[:], ident[:])
                nc.vector.tensor_copy(
                    po[:, j, tch * 512 + c * P: tch * 512 + (c + 1) * P],
                    tpp[:],
                )

            def tp_unit(u):
                for c in range(4):
                    tp_one(u, c)

            def fin_one(tch, c, tag="aux"):
                """Out-proj for one t-block: both 512-wide d-chunks, one store."""
                tsl = slice(tch * 512 + c * P, tch * 512 + (c + 1) * P)
                for dch in range(2):
                    fptag = tag if dch == 0 else ("uacc" if tch == 3 else tag)
                    fp = ps.tile([P, 512], F32, tag=fptag, bufs=2,
                                 name=f"fp{tch}{c}{dch}")
                    dsl = slice(dch * 512, (dch + 1) * 512)
                    for j in range(2):
                        nc.tensor.matmul(fp[:], po[:, j, tsl], wo[:, j, dsl],
                                         start=(j == 0), stop=(j == 1))
                    fo = sb.tile([P, 512], F32, tag="fo", bufs=4,
                                 name=f"fo{tch}{c}{dch}")
                    if tag == "uacc" and dch == 0:
                        nc.scalar.copy(fo[:], fp[:])
                    else:
                        nc.vector.tensor_copy(fo[:], fp[:])
                    nc.sync.dma_start(out_d[tsl, dsl], fo[:])

            # ================= PHASE A: QKV chunks + u00/u01/u10 scores ======
            u00, u01, u10, u11 = (0, 0), (0, 1), (1, 0), (1, 1)
            for u in (u00, u01, u10, u11):
                ebank[u] = {}
            alloc_accs(u00)

            # chunk 0
            qk_part(0, 0, 0)          # q0 m0 -> qT0
            qk_part(0, 1, 0)          # k0 m0 -> kT0
            dma_ft(0, 0)
            dma_ht(1)
            nc.sync.dma_start(bv[:], bv_d[:])
            nc.sync.dma_start(ident[:], id_d[:])
            for st in range(4):
                scores_step(u00, st)
            qk_cast0(0, 0)
            qk_resh(0, 0, 0)
            qk_cast0(1, 0)
            qk_resh(0, 1, 0)
            qk_part(0, 0, 1)
            qk_part(0, 1, 1)
            dma_ft(0, 1)
            for st in range(4):
                scores_step(u01, st)
            qk_cast0(0, 1)
            qk_resh(0, 0, 1)
            qk_cast0(1, 1)
            qk_resh(0, 1, 1)
            # chunks 1-3, serpentine: each chunk's k m-blocks are emitted
            # while the previous groups' exps are still queued, so the
            # bias+reshuffle chain hides; q(n>=1) and v are off-critical.
            qk_part(1, 1, 0)
            nc.sync.dma_start(wv[:], wvT_d.rearrange("(k p) r -> p k r", p=P))
            for n in range(1, 4):
                for st in range(4 * n, 4 * n + 4):
                    scores_step(u00, st)
                if n == 1:
                    qk_cast0(1, 0, n=1)
                    qk_resh(1, 1, 0)
                qk_part(n, 1, 1)
                if n == 1:
                    qk_cast0(1, 1, n=1)
                qk_resh(n, 1, 1)
                qk_part(n, 0, 0)
                qk_part(n, 0, 1)
                qk_resh(n, 0)         # q merged, off critical path
                if n < 3:
                    dma_ht(n + 1)
                    dma_ft(0, n + 1)
                dma_ft(1, n - 1)
                for st in range(4 * n, 4 * n + 4):
                    scores_step(u01, st)
                for st in range(4 * n - 4, 4 * n):
                    scores_step(u10, st)
                for s in range(4 * n - 4, 4 * n):
                    v_stile(s)
                for st in range(4 * n - 4, 4 * n):
                    pv_step(u00, st)
                if n < 3:
                    qk_part(n + 1, 1, 0)
                    qk_resh(n + 1, 1, 0)
                if n == 3:
                    dma_ft(1, 3)

            # phase-A tail: last v group + u00 finish
            for s in range(12, 16):
                v_stile(s)
            for st in range(12, 16):
                pv_step(u00, st)
            nc.sync.dma_start(wo[:], woT_d.rearrange("(m p) d -> p m d", p=P))

            norm_unit(u00)
            tp_unit(u00)

            # ================= PHASE B: weave ===============================
            fresh = [u11, (2, 0), (2, 1), (3, 0), (3, 1)]
            work = [(u10, st) for st in range(12, 16)]
            for w in fresh:
                ebank.setdefault(w, {})
                work += [(w, st) for st in range(ST)]

            # replay queue: (unit, st) in replay order; fresh units appended
            # as their scores complete
            RORD = [0, 1, 2, 4, 5, 6, 7, 8, 10, 11, 12, 13, 3, 9, 14, 15]
            replayq = [(u01, st) for st in RORD]
            replayq += [(u10, st) for st in RORD]
            for w in fresh:
                replayq += [(w, st) for st in RORD]

            # fins become available per tch once both units' tps are done
            finq = []
            tp_done = {u00: True}
            fins_emitted = set()

            ft_sched = {
                (fresh[0], 2): (2, 0), (fresh[0], 6): (2, 1),
                (fresh[0], 10): (2, 2), (fresh[0], 14): (2, 3),
                ((2, 0), 2): (3, 0), ((2, 0), 6): (3, 1),
                ((2, 0), 10): (3, 2), ((2, 0), 14): (3, 3),
            }

            rpi = 0
            sci = 0
            cool = 0
            for (w, st) in work:
                scores_step(w, st)
                sci += 1
                # splice replay PV steps; a step can only replay once its e
                # tile is produced.  After a unit finishes (norm+transpose),
                # pause splicing so the next unit's first PV does not block
                # the PE pipeline while PSUM accumulators drain.
                budget = 2
                if cool > 0:
                    cool -= 1
                    budget = 0
                while budget > 0 and rpi < len(replayq):
                    ru, rst = replayq[rpi]
                    if rst not in ebank.get(ru, {}):
                        break  # not scored yet
                    if rst == 0:
                        alloc_accs(ru)
                    pv_step(ru, rst)
                    budget -= 1
                    rpi += 1
                    if rst == ST - 1:
                        norm_unit(ru)
                        tp_unit(ru)
                        tp_done[ru] = True
                        cool = 2
                        tch = ru[0]
                        other = (tch, 1 - ru[1])
                        if tp_done.get(other) and tch not in fins_emitted:
                            fins_emitted.add(tch)
                            finq += [(tch, c) for c in range(4)]
                        break
                # one fin (both d-chunks) every few scores steps
                if finq and sci % 4 == 0:
                    ftch, fc = finq.pop(0)
                    fin_one(ftch, fc, tag="uacc" if ftch == 3 else "aux")
                if (w, st) in ft_sched:
                    dma_ft(*ft_sched[(w, st)])

            # ---- tail: drain remaining replays, then c-pipelined tp+fin ----
            while rpi < len(replayq):
                ru, rst = replayq[rpi]
                if rst == 0:
                    alloc_accs(ru)
                pv_step(ru, rst)
                rpi += 1
                if rst == ST - 1:
                    norm_unit(ru, tail=True)
                    tch = ru[0]
                    other = (tch, 1 - ru[1])
                    if tp_done.get(other) and tch not in fins_emitted:
                        fins_emitted.add(tch)
                        tp_done[ru] = True
                        for c in range(4):
                            tp_one(ru, c)
                            fin_one(tch, c, tag="uacc")
                    else:
                        tp_unit(ru)
                        tp_done[ru] = True
                while finq:
                    ftch, fc = finq.pop(0)
                    fin_one(ftch, fc, tag="uacc" if ftch == 3 else "aux")
            while finq:
                ftch, fc = finq.pop(0)
                fin_one(ftch, fc, tag="uacc" if ftch == 3 else "aux")

    return nc


_NC = None
_LAST_RESULT = None


def _get_nc():
    global _NC
    if _NC is None:
        _NC = build_bass()
        if not _NC.is_finalized():
            _NC.finalize()
    return _NC


def kernel(hidden_states, focused_attention, Wq, bq, Wk, bk, Wv, bv, Wo, bo):
    bf = ml_dtypes.bfloat16
    hT = [np.ascontiguousarray(hidden_states[b].T).astype(bf) for b in range(B)]
    fT = [np.ascontiguousarray(focused_attention[b].T).astype(bf) for b in range(B)]

    in_maps = []
    for c in range(N_CORES):
        b, g = divmod(c, 4)
        rows = slice(g * R, (g + 1) * R)
        in_maps.append({
            "hT": hT[b],
            "fT": fT[b],
            "wqT": np.ascontiguousarray((Wq[rows] * (SCALING * QSC)).T).astype(bf),
            "wkT": np.ascontiguousarray((Wk[rows] * KSC).T).astype(bf),
            "wvT": np.ascontiguousarray(Wv[rows].T).astype(bf),
            "woT": np.ascontiguousarray(Wo[:, rows].T).astype(bf),
            "bq": np.ascontiguousarray(
                (bq[rows] * (SCALING * QSC))[:, None]).astype(np.float32),
            "bk": np.ascontiguousarray((bk[rows] * KSC)[:, None]).astype(np.float32),
            "bv": np.ascontiguousarray(bv[rows][None, :]).astype(bf),
            "ident": np.eye(P, dtype=bf),
        })

    res = run_bass_kernel_spmd(_get_nc(), in_maps, list(range(N_CORES)))
    global _LAST_RESULT
    _LAST_RESULT = res
    out = np.zeros((B, T, D), dtype=np.float32)
    for c in range(N_CORES):
        out[c // 4] += res.results[c]["out_partial"]
    out += np.asarray(bo, dtype=np.float32)[None, None, :]
    return out


# revision 6
# speedup vs baseline: 1.0051x; 1.0011x over previous
"""BartAttention (focused-attention variant) Trainium2 Bass kernel, v2.

Problem (hardcoded): B=2, T=2048, D=1024, H=16 heads, hd=64.
  q = (h @ Wq.T + bq) * hd**-0.5 ; k = h @ Wk.T + bk ; v = h @ Wv.T + bv
  scores = q @ k.T per head ; e = f * exp(scores) ; attn = e / rowsum(e)
  out = (attn @ v) @ Wo.T + bo

Sharding over 8 cores: batch (2) x head-group (4 groups of 4 heads); host
sums the 4 partial out-projections per batch and adds bo.

Per-core design (4 heads = 2 pairs j=0,1; ACT-exp is the bottleneck engine
at ~134us busy, everything else is scheduled to hide under it):
  - q/k projections bf16; q,k pre-scaled x16/x8 on the host and stored as
    fp8e4m3 in an hd-split [32 partitions, half, head, t] layout via
    SBUF->SBUF reshuffle DMAs
  - scores: fp8 DoubleRow matmuls (two 32-row contraction tiles per
    instruction -> 256 cycles per [128,512] block, 2x over bf16);
    sc = 128*q.k in PSUM f32; the first 4 s-tile groups of the (0,j) units
    use a bf16 path instead so the exp stream starts before any reshuffle
  - exp: ACT only, e = exp(sc/128) on [128,1024] tiles (2 heads);
    e *= fT in place on DVE (a few steps per unit go to GPSIMD to keep
    DVE under ACT)
  - PV e-stationary: acc[t-block, 65] += e_block.T @ [v|1]; N=65 matmuls;
    column 64 accumulates rowsum(e), so normalization is a per-partition
    reciprocal + tensor_scalar multiply (no broadcast matmuls)
  - po blocks transposed to [r, t] with PE transpose + DVE copy;
    out-proj per (t-block, d-chunk) accumulates both head pairs, DVE/ACT
    copies PSUM->SBUF, DMA out
  - schedule: QKV chunks serpentined with the scores of units (0,0), (0,1)
    and (1,0) (PV deferred into an SBUF e-backlog) so ACT never starves in
    phase A; phase B replays each unit's PV one unit behind the fresh
    scores stream, with out-projections and transposes woven between.
"""

import numpy as np
import ml_dtypes

import concourse.bass as bass
import concourse.bacc as bacc
import concourse.mybir as mybir
from concourse.tile import TileContext
from concourse.bass_utils import run_bass_kernel_spmd

BF16 = mybir.dt.bfloat16
F32 = mybir.dt.float32
F8 = mybir.dt.float8e4
AF = mybir.ActivationFunctionType
DR = mybir.MatmulPerfMode.DoubleRow

B, T, D = 2, 2048, 1024
H, HD = 16, 64
HG = 4               # heads per core
R = HG * HD          # 256 rows per core
SCALING = HD ** -0.5
N_CORES = 8

P = 128
KT = D // P          # 8 k-tiles for QKV contraction
NCH = T // 512       # 4 t-chunks
ST = T // P          # 16 s-tiles

QSC, KSC = 16.0, 8.0         # fp8 pre-scales for q and k
EXP_SCALE = 1.0 / (QSC * KSC)


def build_bass():
    nc = bacc.Bacc()

    hT_d = nc.declare_dram_parameter("hT", [D, T], BF16, isOutput=False)
    fT_d = nc.declare_dram_parameter("fT", [T, T], BF16, isOutput=False)
    wqT_d = nc.declare_dram_parameter("wqT", [D, R], BF16, isOutput=False)
    wkT_d = nc.declare_dram_parameter("wkT", [D, R], BF16, isOutput=False)
    wvT_d = nc.declare_dram_parameter("wvT", [D, R], BF16, isOutput=False)
    woT_d = nc.declare_dram_parameter("woT", [R, D], BF16, isOutput=False)
    bq_d = nc.declare_dram_parameter("bq", [R, 1], F32, isOutput=False)
    bk_d = nc.declare_dram_parameter("bk", [R, 1], F32, isOutput=False)
    bv_d = nc.declare_dram_parameter("bv", [1, R], BF16, isOutput=False)
    id_d = nc.declare_dram_parameter("ident", [P, P], BF16, isOutput=False)
    out_d = nc.declare_dram_parameter("out_partial", [T, D], F32, isOutput=True)

    hT_r = hT_d.rearrange("(k p) t -> p k t", p=P)
    fT_r = fT_d.rearrange("(s p) t -> p s t", p=P)

    with TileContext(nc) as tc:
        with (
            nc.allow_low_precision(reason="bf16/fp8 pipeline is intentional"),
            tc.tile_pool(name="sb", bufs=1) as sb,
            tc.tile_pool(name="ps", bufs=1, space="PSUM") as ps,
        ):
            # ---- persistent SBUF ----
            wq = sb.tile([P, KT, R], BF16)
            wk = sb.tile([P, KT, R], BF16)
            wv = sb.tile([P, KT, R], BF16)
            wo = sb.tile([P, 2, D], BF16)
            bq = sb.tile([P, 2], F32)
            bk = sb.tile([P, 2], F32)
            bv = sb.tile([1, R], BF16)
            ident = sb.tile([P, P], BF16)
            ones_r = sb.tile([1, P], BF16)
            q8s = sb.tile([32, 2, HG, T], F8)   # [p32, half, head, t]
            k8s = sb.tile([32, 2, HG, T], F8)
            vsb = sb.tile([P, ST, HG, HD + 1], BF16)
            qT0 = sb.tile([P, 2, 512], BF16)    # chunk-0 q bf16 (ramp path)
            kT0 = sb.tile([P, 2, 1024], BF16)   # chunks 0-1 k bf16 (seam path)
            po = sb.tile([P, 2, T], BF16)       # out-proj lhsT [r, (tch c t)]

            nc.vector.memset(ones_r[:], 1.0)
            nc.vector.memset(vsb[:, :, :, HD:HD + 1], 1.0)

            # PE warm-up: burn the p-state ramp on junk matmuls during DMA
            for i in range(24):
                wrm = ps.tile([P, P], F32, tag="sc", bufs=2, name=f"warm{i}")
                nc.tensor.matmul(wrm[:], ones_r[:], ones_r[:],
                                 start=True, stop=True)

            # ---- initial DMAs ----
            nc.sync.dma_start(wq[:], wqT_d.rearrange("(k p) r -> p k r", p=P))
            ht = {}

            def dma_ht(n):
                t = sb.tile([P, KT, 512], BF16, tag="ht", bufs=3, name=f"ht{n}")
                nsl = slice(n * 512, (n + 1) * 512)
                nc.sync.dma_start(t[:, 0:4, :], hT_r[:, 0:4, nsl])
                nc.sync.dma_start(t[:, 4:8, :], hT_r[:, 4:8, nsl])
                ht[n] = t

            ht0 = sb.tile([P, KT, 512], BF16, tag="ht", bufs=3, name="ht0")
            nc.sync.dma_start(ht0[:, 0:4, :], hT_r[:, 0:4, 0:512])
            nc.sync.dma_start(wk[:], wkT_d.rearrange("(k p) r -> p k r", p=P))
            nc.sync.dma_start(ht0[:, 4:8, :], hT_r[:, 4:8, 0:512])
            ht[0] = ht0
            nc.sync.dma_start(bq[:], bq_d.rearrange("(m p) one -> p (m one)", p=P))
            nc.sync.dma_start(bk[:], bk_d.rearrange("(m p) one -> p (m one)", p=P))

            fts = {}

            def dma_ft(tch, g):
                t = sb.tile([P, 4, 512], BF16, tag="ft", bufs=8,
                            name=f"ft{tch}g{g}")
                nc.sync.dma_start(
                    t[:], fT_r[:, 4 * g:4 * g + 4, tch * 512:(tch + 1) * 512]
                )
                fts[(tch, g)] = t

            # ---- helpers ----
            st8s = {}

            def qk_part(n, tens, m, cols=None):
                """q or k projection matmuls + bias for chunk n, m-block.
                cols: optional (lo, hi) sub-range of the 512 chunk columns."""
                w_sb, b_sb = (wq, bq) if tens == 0 else (wk, bk)
                lo, hi = cols if cols else (0, 512)
                nm = f"{'qk'[tens]}{n}m{m}c{lo}"
                acc = ps.tile([P, hi - lo], F32, tag="aux", bufs=2,
                              name=f"a{nm}")
                for kk in range(KT):
                    nc.tensor.matmul(
                        acc[:], w_sb[:, kk, m * P:(m + 1) * P],
                        ht[n][:, kk, lo:hi],
                        start=(kk == 0), stop=(kk == KT - 1),
                    )
                if n == 0 or (n == 1 and tens == 1):
                    dst = qT0 if tens == 0 else kT0
                    off = 0 if n == 0 else 512
                    nc.vector.tensor_scalar_add(dst[:, m, off + lo:off + hi],
                                                acc[:], b_sb[:, m:m + 1])
                    return
                if (tens, n) not in st8s:
                    st8s[(tens, n)] = sb.tile([P, 2, 512], F8, tag="st8",
                                              bufs=4, name=f"s{'qk'[tens]}{n}")
                nc.vector.tensor_scalar_add(st8s[(tens, n)][:, m, lo:hi],
                                            acc[:], b_sb[:, m:m + 1])

            def qk_cast0(tens, m, n=0):
                if (tens, n) not in st8s:
                    st8s[(tens, n)] = sb.tile([P, 2, 512], F8, tag="st8",
                                              bufs=4, name=f"s{'qk'[tens]}{n}")
                src_t = qT0 if tens == 0 else kT0
                off = 0 if n == 0 else 512
                nc.vector.tensor_copy(st8s[(tens, n)][:, m, :],
                                      src_t[:, m, off:off + 512])

            def qk_resh(n, tens, m=None, cols=None):
                """Reshuffle chunk n into the [32, half, head, t] hd-split
                layout. m=None: both m-blocks in 4 DMAs of [32, 2, 512]
                (m via stride-2 head dim); m=int: that m-block only."""
                dst = q8s if tens == 0 else k8s
                st8 = st8s[(tens, n)]
                lo, hi = cols if cols else (0, 512)
                nsl = slice(n * 512 + lo, n * 512 + hi)
                for half in range(2):
                    for hm in range(2):
                        src_p = slice(64 * hm + 32 * half, 64 * hm + 32 * half + 32)
                        if m is None:
                            nc.sync.dma_start(dst[:, half, hm::2, nsl],
                                              st8[src_p, :, lo:hi])
                        else:
                            nc.sync.dma_start(dst[:, half, 2 * m + hm, nsl],
                                              st8[src_p, m, lo:hi])

            def v_stile(s):
                acc = ps.tile([P, R], F32, tag="aux", bufs=2, name=f"vacc{s}")
                for kk in range(KT):
                    nc.tensor.matmul(
                        acc[:], ht[s // 4][:, kk, (s % 4) * P:(s % 4 + 1) * P],
                        wv[:, kk, :], start=(kk == 0), stop=False,
                    )
                nc.tensor.matmul(acc[:], ones_r[:], bv[:], start=False, stop=True)
                nc.vector.tensor_copy(
                    vsb[:, s, :, 0:HD],
                    acc[:].rearrange("p (h d) -> p h d", h=HG),
                )

            ebank = {}    # u -> {st: e_tile} pending PV
            accs = {}     # u -> (acc_a, acc_b)
            poTs = {}     # (u, c) -> poT tile

            def scores_step(u, st):
                tch, j = u
                sc = ps.tile([P, 1024], F32, tag="sc", bufs=2,
                             name=f"sc{tch}{j}_{st}")
                tsl = slice(tch * 512, (tch + 1) * 512)
                ssl = slice(st * P, (st + 1) * P)
                for a in range(2):
                    h = 2 * j + a
                    if tch == 0 and st < 8:
                        rows = slice(a * HD, (a + 1) * HD)
                        nc.tensor.matmul(
                            sc[:, a * 512:(a + 1) * 512],
                            kT0[rows, j, st * P:(st + 1) * P],
                            qT0[rows, j, :],
                            start=True, stop=True,
                        )
                    else:
                        nc.tensor.matmul(
                            sc[:, a * 512:(a + 1) * 512],
                            k8s[:, :, h, ssl], q8s[:, :, h, tsl],
                            start=True, stop=True, perf_mode=DR,
                        )
                e = sb.tile([P, 1024], BF16, tag="e", bufs=32,
                            name=f"e{tch}{j}_{st}")
                nc.scalar.activation(e[:], sc[:], AF.Exp, scale=EXP_SCALE)
                ftt = fts[(tch, st // 4)]
                eng = nc.gpsimd if st in (3, 9, 14) else nc.vector
                for a in range(2):
                    half = slice(a * 512, (a + 1) * 512)
                    eng.tensor_mul(e[:, half], e[:, half],
                                   ftt[:, st % 4, :])
                ebank[u][st] = e

            def alloc_accs(u):
                accs[u] = tuple(
                    ps.tile([P, 4, HD + 1], F32, tag="uacc", bufs=2,
                            name=f"acc{u[0]}{u[1]}{a}")
                    for a in range(2)
                )

            def pv_step(u, st, first=None, last=None):
                tch, j = u
                first = (st == 0) if first is None else first
                last = (st == ST - 1) if last is None else last
                e = ebank[u].pop(st)
                for a in range(2):
                    acc = accs[u][a]
                    for c in range(4):
                        nc.tensor.matmul(
                            acc[:, c, :],
                            e[:, a * 512 + c * P:a * 512 + (c + 1) * P],
                            vsb[:, st, 2 * j + a, :],
                            start=(first and c == 0),
                            stop=(last and c == 3),
                            skip_group_check=True,
                        )

            def norm_unit(u, tail=False):
                for a in range(2):
                    acc = accs[u][a]
                    rc = sb.tile([P, 4, 1], F32, tag="rc", bufs=4,
                                 name=f"rc{u[0]}{u[1]}{a}")
                    nc.vector.reciprocal(rc[:], acc[:, :, HD:HD + 1])
                    for c in range(4):
                        if (u, c) not in poTs:
                            poTs[(u, c)] = sb.tile(
                                [P, P], BF16, tag="pt", bufs=8,
                                name=f"pt{u[0]}{u[1]}{c}")
                        if tail and a == 1:
                            nc.scalar.mul(
                                poTs[(u, c)][:, a * HD:(a + 1) * HD],
                                acc[:, c, 0:HD], rc[:, c, :],
                            )
                        else:
                            nc.vector.tensor_scalar_mul(
                                poTs[(u, c)][:, a * HD:(a + 1) * HD],
                                acc[:, c, 0:HD], rc[:, c, :],
                            )

            def tp_one(u, c):
                tch, j = u
                tpp = ps.tile([P, P], BF16, tag="aux", bufs=2,
                              name=f"tpp{tch}{j}{c}")
                nc.tensor.transpose(tpp[:], poTs.pop((u, c))[:], ident[:])
                nc.vector.tensor_copy(
                    po[:, j, tch * 512 + c * P: tch * 512 + (c + 1) * P],
                    tpp[:],
                )

            def tp_unit(u):
                for c in range(4):
                    tp_one(u, c)

            def fin_one(tch, c, tag="aux"):
                """Out-proj for one t-block: both 512-wide d-chunks, one store."""
                tsl = slice(tch * 512 + c * P, tch * 512 + (c + 1) * P)
                for dch in range(2):
                    fptag = tag if dch == 0 else ("uacc" if tch == 3 else tag)
                    fp = ps.tile([P, 512], F32, tag=fptag, bufs=2,
                                 name=f"fp{tch}{c}{dch}")
                    dsl = slice(dch * 512, (dch + 1) * 512)
                    for j in range(2):
                        nc.tensor.matmul(fp[:], po[:, j, tsl], wo[:, j, dsl],
                                         start=(j == 0), stop=(j == 1))
                    fo = sb.tile([P, 512], F32, tag="fo", bufs=4,
                                 name=f"fo{tch}{c}{dch}")
                    if tag == "uacc" and dch == 0:
                        nc.scalar.copy(fo[:], fp[:])
                    else:
                        nc.vector.tensor_copy(fo[:], fp[:])
                    nc.sync.dma_start(out_d[tsl, dsl], fo[:])

            # ================= PHASE A: QKV chunks + u00/u01/u10 scores ======
            u00, u01, u10, u11 = (0, 0), (0, 1), (1, 0), (1, 1)
            for u in (u00, u01, u10, u11):
                ebank[u] = {}
            alloc_accs(u00)

            # chunk 0
            qk_part(0, 0, 0)          # q0 m0 -> qT0
            qk_part(0, 1, 0)          # k0 m0 -> kT0
            dma_ft(0, 0)
            dma_ht(1)
            nc.sync.dma_start(bv[:], bv_d[:])
            nc.sync.dma_start(ident[:], id_d[:])
            for st in range(4):
                scores_step(u00, st)
            qk_cast0(0, 0)
            qk_resh(0, 0, 0)
            qk_cast0(1, 0)
            qk_resh(0, 1, 0)
            qk_part(0, 0, 1)
            qk_part(0, 1, 1)
            dma_ft(0, 1)
            for st in range(4):
                scores_step(u01, st)
            qk_cast0(0, 1)
            qk_resh(0, 0, 1)
            qk_cast0(1, 1)
            qk_resh(0, 1, 1)
            # chunks 1-3, serpentine: each chunk's k m-blocks are emitted
            # while the previous groups' exps are still queued, so the
            # bias+reshuffle chain hides; q(n>=1) and v are off-critical.
            qk_part(1, 1, 0)
            nc.sync.dma_start(wv[:], wvT_d.rearrange("(k p) r -> p k r", p=P))
            for n in range(1, 4):
                for st in range(4 * n, 4 * n + 4):
                    scores_step(u00, st)
                if n == 1:
                    qk_cast0(1, 0, n=1)
                    qk_resh(1, 1, 0)
                qk_part(n, 1, 1)
                if n == 1:
                    qk_cast0(1, 1, n=1)
                qk_resh(n, 1, 1)
                qk_part(n, 0, 0)
                qk_part(n, 0, 1)
                qk_resh(n, 0)         # q merged, off critical path
                if n < 3:
                    dma_ht(n + 1)
                    dma_ft(0, n + 1)
                dma_ft(1, n - 1)
                for st in range(4 * n, 4 * n + 4):
                    scores_step(u01, st)
                for st in range(4 * n - 4, 4 * n):
                    scores_step(u10, st)
                for s in range(4 * n - 4, 4 * n):
                    v_stile(s)
                for st in range(4 * n - 4, 4 * n):
                    pv_step(u00, st)
                if n < 3:
                    qk_part(n + 1, 1, 0)
                    qk_resh(n + 1, 1, 0)
                if n == 3:
                    dma_ft(1, 3)

            # phase-A tail: last v group + u00 finish
            for s in range(12, 16):
                v_stile(s)
            for st in range(12, 16):
                pv_step(u00, st)
            nc.sync.dma_start(wo[:], woT_d.rearrange("(m p) d -> p m d", p=P))

            norm_unit(u00)
            tp_unit(u00)

            # ================= PHASE B: weave ===============================
            fresh = [u11, (2, 0), (2, 1), (3, 0), (3, 1)]
            work = [(u10, st) for st in range(12, 16)]
            for w in fresh:
                ebank.setdefault(w, {})
                work += [(w, st) for st in range(ST)]

            # replay queue: (unit, st) in replay order; fresh units appended
            # as their scores complete
            RORD = [0, 1, 2, 4, 5, 6, 7, 8, 10, 11, 12, 13, 3, 9, 14, 15]
            replayq = [(u01, st) for st in RORD]
            replayq += [(u10, st) for st in RORD]
            for w in fresh:
                replayq += [(w, st) for st in RORD]

            # fins become available per tch once both units' tps are done
            finq = []
            tp_done = {u00: True}
            fins_emitted = set()

            ft_sched = {
                (fresh[0], 2): (2, 0), (fresh[0], 6): (2, 1),
                (fresh[0], 10): (2, 2), (fresh[0], 14): (2, 3),
                ((2, 0), 2): (3, 0), ((2, 0), 6): (3, 1),
                ((2, 0), 10): (3, 2), ((2, 0), 14): (3, 3),
            }

            rpi = 0
            sci = 0
            cool = 0
            for (w, st) in work:
                scores_step(w, st)
                sci += 1
                # splice replay PV steps; a step can only replay once its e
                # tile is produced.  After a unit finishes (norm+transpose),
                # pause splicing so the next unit's first PV does not block
                # the PE pipeline while PSUM accumulators drain.
                budget = 2
                if cool > 0:
                    cool -= 1
                    budget = 0
                while budget > 0 and rpi < len(replayq):
                    ru, rst = replayq[rpi]
                    if rst not in ebank.get(ru, {}):
                        break  # not scored yet
                    if rst == 0:
                        alloc_accs(ru)
                    pv_step(ru, rst)
                    budget -= 1
                    rpi += 1
                    if rst == ST - 1:
                        norm_unit(ru)
                        tp_unit(ru)
                        tp_done[ru] = True
                        cool = 2
                        tch = ru[0]
                        other = (tch, 1 - ru[1])
                        if tp_done.get(other) and tch not in fins_emitted:
                            fins_emitted.add(tch)
                            finq += [(tch, c) for c in range(4)]
                        break
                # one fin (both d-chunks) every few scores steps
                if finq and sci % 3 == 0:
                    ftch, fc = finq.pop(0)
                    fin_one(ftch, fc, tag="uacc" if ftch == 3 else "aux")
                if (w, st) in ft_sched:
                    dma_ft(*ft_sched[(w, st)])

            # ---- tail: drain remaining replays, then c-pipelined tp+fin ----
            while rpi < len(replayq):
                ru, rst = replayq[rpi]
                if rst == 0:
                    alloc_accs(ru)
                pv_step(ru, rst)
                rpi += 1
                if rst == ST - 1:
                    norm_unit(ru, tail=True)
                    tch = ru[0]
                    other = (tch, 1 - ru[1])
                    if tp_done.get(other) and tch not in fins_emitted:
                        fins_emitted.add(tch)
                        tp_done[ru] = True
                        for c in range(4):
                            tp_one(ru, c)
                            fin_one(tch, c, tag="uacc")
                    else:
                        tp_unit(ru)
                        tp_done[ru] = True
                while finq:
                    ftch, fc = finq.pop(0)
                    fin_one(ftch, fc, tag="uacc" if ftch == 3 else "aux")
            while finq:
                ftch, fc = finq.pop(0)
                fin_one(ftch, fc, tag="uacc" if ftch == 3 else "aux")

    return nc


_NC = None
_LAST_RESULT = None


def _get_nc():
    global _NC
    if _NC is None:
        _NC = build_bass()
        if not _NC.is_finalized():
            _NC.finalize()
    return _NC


def kernel(hidden_states, focused_attention, Wq, bq, Wk, bk, Wv, bv, Wo, bo):
    bf = ml_dtypes.bfloat16
    hT = [np.ascontiguousarray(hidden_states[b].T).astype(bf) for b in range(B)]
    fT = [np.ascontiguousarray(focused_attention[b].T).astype(bf) for b in range(B)]

    in_maps = []
    for c in range(N_CORES):
        b, g = divmod(c, 4)
        rows = slice(g * R, (g + 1) * R)
        in_maps.append({
            "hT": hT[b],
            "fT": fT[b],
            "wqT": np.ascontiguousarray((Wq[rows] * (SCALING * QSC)).T).astype(bf),
            "wkT": np.ascontiguousarray((Wk[rows] * KSC).T).astype(bf),
            "wvT": np.ascontiguousarray(Wv[rows].T).astype(bf),
            "woT": np.ascontiguousarray(Wo[:, rows].T).astype(bf),
            "bq": np.ascontiguousarray(
                (bq[rows] * (SCALING * QSC))[:, None]).astype(np.float32),
            "bk": np.ascontiguousarray((bk[rows] * KSC)[:, None]).astype(np.float32),
            "bv": np.ascontiguousarray(bv[rows][None, :]).astype(bf),
            "ident": np.eye(P, dtype=bf),
        })

    res = run_bass_kernel_spmd(_get_nc(), in_maps, list(range(N_CORES)))
    global _LAST_RESULT
    _LAST_RESULT = res
    out = np.zeros((B, T, D), dtype=np.float32)
    for c in range(N_CORES):
        out[c // 4] += res.results[c]["out_partial"]
    out += np.asarray(bo, dtype=np.float32)[None, None, :]
    return out


# revision 7
# speedup vs baseline: 1.0068x; 1.0017x over previous
"""BartAttention (focused-attention variant) Trainium2 Bass kernel, v2.

Problem (hardcoded): B=2, T=2048, D=1024, H=16 heads, hd=64.
  q = (h @ Wq.T + bq) * hd**-0.5 ; k = h @ Wk.T + bk ; v = h @ Wv.T + bv
  scores = q @ k.T per head ; e = f * exp(scores) ; attn = e / rowsum(e)
  out = (attn @ v) @ Wo.T + bo

Sharding over 8 cores: batch (2) x head-group (4 groups of 4 heads); host
sums the 4 partial out-projections per batch and adds bo.

Per-core design (4 heads = 2 pairs j=0,1; ACT-exp is the bottleneck engine
at ~134us busy, everything else is scheduled to hide under it):
  - q/k projections bf16; q,k pre-scaled x16/x8 on the host and stored as
    fp8e4m3 in an hd-split [32 partitions, half, head, t] layout via
    SBUF->SBUF reshuffle DMAs
  - scores: fp8 DoubleRow matmuls (two 32-row contraction tiles per
    instruction -> 256 cycles per [128,512] block, 2x over bf16);
    sc = 128*q.k in PSUM f32; the first 4 s-tile groups of the (0,j) units
    use a bf16 path instead so the exp stream starts before any reshuffle
  - exp: ACT only, e = exp(sc/128) on [128,1024] tiles (2 heads);
    e *= fT in place on DVE (a few steps per unit go to GPSIMD to keep
    DVE under ACT)
  - PV e-stationary: acc[t-block, 65] += e_block.T @ [v|1]; N=65 matmuls;
    column 64 accumulates rowsum(e), so normalization is a per-partition
    reciprocal + tensor_scalar multiply (no broadcast matmuls)
  - po blocks transposed to [r, t] with PE transpose + DVE copy;
    out-proj per (t-block, d-chunk) accumulates both head pairs, DVE/ACT
    copies PSUM->SBUF, DMA out
  - schedule: QKV chunks serpentined with the scores of units (0,0), (0,1)
    and (1,0) (PV deferred into an SBUF e-backlog) so ACT never starves in
    phase A; phase B replays each unit's PV one unit behind the fresh
    scores stream, with out-projections and transposes woven between.
"""

import numpy as np
import ml_dtypes

import concourse.bass as bass
import concourse.bacc as bacc
import concourse.mybir as mybir
from concourse.tile import TileContext
from concourse.bass_utils import run_bass_kernel_spmd

BF16 = mybir.dt.bfloat16
F32 = mybir.dt.float32
F8 = mybir.dt.float8e4
AF = mybir.ActivationFunctionType
DR = mybir.MatmulPerfMode.DoubleRow

B, T, D = 2, 2048, 1024
H, HD = 16, 64
HG = 4               # heads per core
R = HG * HD          # 256 rows per core
SCALING = HD ** -0.5
N_CORES = 8

P = 128
KT = D // P          # 8 k-tiles for QKV contraction
NCH = T // 512       # 4 t-chunks
ST = T // P          # 16 s-tiles

QSC, KSC = 16.0, 8.0         # fp8 pre-scales for q and k
EXP_SCALE = 1.0 / (QSC * KSC)


def build_bass():
    nc = bacc.Bacc()

    hT_d = nc.declare_dram_parameter("hT", [D, T], BF16, isOutput=False)
    fT_d = nc.declare_dram_parameter("fT", [T, T], BF16, isOutput=False)
    wqT_d = nc.declare_dram_parameter("wqT", [D, R], BF16, isOutput=False)
    wkT_d = nc.declare_dram_parameter("wkT", [D, R], BF16, isOutput=False)
    wvT_d = nc.declare_dram_parameter("wvT", [D, R], BF16, isOutput=False)
    woT_d = nc.declare_dram_parameter("woT", [R, D], BF16, isOutput=False)
    bq_d = nc.declare_dram_parameter("bq", [R, 1], F32, isOutput=False)
    bk_d = nc.declare_dram_parameter("bk", [R, 1], F32, isOutput=False)
    bv_d = nc.declare_dram_parameter("bv", [1, R], BF16, isOutput=False)
    id_d = nc.declare_dram_parameter("ident", [P, P], BF16, isOutput=False)
    out_d = nc.declare_dram_parameter("out_partial", [T, D], F32, isOutput=True)

    hT_r = hT_d.rearrange("(k p) t -> p k t", p=P)
    fT_r = fT_d.rearrange("(s p) t -> p s t", p=P)

    with TileContext(nc) as tc:
        with (
            nc.allow_low_precision(reason="bf16/fp8 pipeline is intentional"),
            tc.tile_pool(name="sb", bufs=1) as sb,
            tc.tile_pool(name="ps", bufs=1, space="PSUM") as ps,
        ):
            # ---- persistent SBUF ----
            wq = sb.tile([P, KT, R], BF16)
            wk = sb.tile([P, KT, R], BF16)
            wv = sb.tile([P, KT, R], BF16)
            wo = sb.tile([P, 2, D], BF16)
            bq = sb.tile([P, 2], F32)
            bk = sb.tile([P, 2], F32)
            bv = sb.tile([1, R], BF16)
            ident = sb.tile([P, P], BF16)
            ones_r = sb.tile([1, P], BF16)
            q8s = sb.tile([32, 2, HG, T], F8)   # [p32, half, head, t]
            k8s = sb.tile([32, 2, HG, T], F8)
            vsb = sb.tile([P, ST, HG, HD + 1], BF16)
            qT0 = sb.tile([P, 2, 512], BF16)    # chunk-0 q bf16 (ramp path)
            kT0 = sb.tile([P, 2, 1024], BF16)   # chunks 0-1 k bf16 (seam path)
            po = sb.tile([P, 2, T], BF16)       # out-proj lhsT [r, (tch c t)]

            nc.vector.memset(ones_r[:], 1.0)
            nc.vector.memset(vsb[:, :, :, HD:HD + 1], 1.0)

            # PE warm-up: burn the p-state ramp on junk matmuls during DMA
            for i in range(24):
                wrm = ps.tile([P, P], F32, tag="sc", bufs=2, name=f"warm{i}")
                nc.tensor.matmul(wrm[:], ones_r[:], ones_r[:],
                                 start=True, stop=True)

            # ---- initial DMAs ----
            wq_r = wqT_d.rearrange("(k p) r -> p k r", p=P)
            wk_r = wkT_d.rearrange("(k p) r -> p k r", p=P)
            nc.sync.dma_start(wq[:, 0:4, :], wq_r[:, 0:4, :])
            ht = {}

            def dma_ht(n):
                t = sb.tile([P, KT, 512], BF16, tag="ht", bufs=3, name=f"ht{n}")
                nsl = slice(n * 512, (n + 1) * 512)
                nc.sync.dma_start(t[:, 0:4, :], hT_r[:, 0:4, nsl])
                nc.sync.dma_start(t[:, 4:8, :], hT_r[:, 4:8, nsl])
                ht[n] = t

            ht0 = sb.tile([P, KT, 512], BF16, tag="ht", bufs=3, name="ht0")
            nc.sync.dma_start(ht0[:, 0:4, :], hT_r[:, 0:4, 0:512])
            nc.sync.dma_start(wk[:, 0:4, :], wk_r[:, 0:4, :])
            nc.sync.dma_start(wq[:, 4:8, :], wq_r[:, 4:8, :])
            nc.sync.dma_start(ht0[:, 4:8, :], hT_r[:, 4:8, 0:512])
            nc.sync.dma_start(wk[:, 4:8, :], wk_r[:, 4:8, :])
            ht[0] = ht0
            nc.sync.dma_start(bq[:], bq_d.rearrange("(m p) one -> p (m one)", p=P))
            nc.sync.dma_start(bk[:], bk_d.rearrange("(m p) one -> p (m one)", p=P))

            fts = {}

            def dma_ft(tch, g):
                t = sb.tile([P, 4, 512], BF16, tag="ft", bufs=8,
                            name=f"ft{tch}g{g}")
                nc.sync.dma_start(
                    t[:], fT_r[:, 4 * g:4 * g + 4, tch * 512:(tch + 1) * 512]
                )
                fts[(tch, g)] = t

            # ---- helpers ----
            st8s = {}

            def qk_part(n, tens, m, cols=None):
                """q or k projection matmuls + bias for chunk n, m-block.
                cols: optional (lo, hi) sub-range of the 512 chunk columns."""
                w_sb, b_sb = (wq, bq) if tens == 0 else (wk, bk)
                lo, hi = cols if cols else (0, 512)
                nm = f"{'qk'[tens]}{n}m{m}c{lo}"
                acc = ps.tile([P, hi - lo], F32, tag="aux", bufs=2,
                              name=f"a{nm}")
                for kk in range(KT):
                    nc.tensor.matmul(
                        acc[:], w_sb[:, kk, m * P:(m + 1) * P],
                        ht[n][:, kk, lo:hi],
                        start=(kk == 0), stop=(kk == KT - 1),
                    )
                if n == 0 or (n == 1 and tens == 1):
                    dst = qT0 if tens == 0 else kT0
                    off = 0 if n == 0 else 512
                    nc.vector.tensor_scalar_add(dst[:, m, off + lo:off + hi],
                                                acc[:], b_sb[:, m:m + 1])
                    return
                if (tens, n) not in st8s:
                    st8s[(tens, n)] = sb.tile([P, 2, 512], F8, tag="st8",
                                              bufs=4, name=f"s{'qk'[tens]}{n}")
                nc.vector.tensor_scalar_add(st8s[(tens, n)][:, m, lo:hi],
                                            acc[:], b_sb[:, m:m + 1])

            def qk_cast0(tens, m, n=0):
                if (tens, n) not in st8s:
                    st8s[(tens, n)] = sb.tile([P, 2, 512], F8, tag="st8",
                                              bufs=4, name=f"s{'qk'[tens]}{n}")
                src_t = qT0 if tens == 0 else kT0
                off = 0 if n == 0 else 512
                nc.vector.tensor_copy(st8s[(tens, n)][:, m, :],
                                      src_t[:, m, off:off + 512])

            def qk_resh(n, tens, m=None, cols=None):
                """Reshuffle chunk n into the [32, half, head, t] hd-split
                layout. m=None: both m-blocks in 4 DMAs of [32, 2, 512]
                (m via stride-2 head dim); m=int: that m-block only."""
                dst = q8s if tens == 0 else k8s
                st8 = st8s[(tens, n)]
                lo, hi = cols if cols else (0, 512)
                nsl = slice(n * 512 + lo, n * 512 + hi)
                for half in range(2):
                    for hm in range(2):
                        src_p = slice(64 * hm + 32 * half, 64 * hm + 32 * half + 32)
                        if m is None:
                            nc.sync.dma_start(dst[:, half, hm::2, nsl],
                                              st8[src_p, :, lo:hi])
                        else:
                            nc.sync.dma_start(dst[:, half, 2 * m + hm, nsl],
                                              st8[src_p, m, lo:hi])

            def v_stile(s):
                acc = ps.tile([P, R], F32, tag="aux", bufs=2, name=f"vacc{s}")
                for kk in range(KT):
                    nc.tensor.matmul(
                        acc[:], ht[s // 4][:, kk, (s % 4) * P:(s % 4 + 1) * P],
                        wv[:, kk, :], start=(kk == 0), stop=False,
                    )
                nc.tensor.matmul(acc[:], ones_r[:], bv[:], start=False, stop=True)
                nc.vector.tensor_copy(
                    vsb[:, s, :, 0:HD],
                    acc[:].rearrange("p (h d) -> p h d", h=HG),
                )

            ebank = {}    # u -> {st: e_tile} pending PV
            accs = {}     # u -> (acc_a, acc_b)
            poTs = {}     # (u, c) -> poT tile

            def scores_step(u, st):
                tch, j = u
                sc = ps.tile([P, 1024], F32, tag="sc", bufs=2,
                             name=f"sc{tch}{j}_{st}")
                tsl = slice(tch * 512, (tch + 1) * 512)
                ssl = slice(st * P, (st + 1) * P)
                for a in range(2):
                    h = 2 * j + a
                    if tch == 0 and st < 8:
                        rows = slice(a * HD, (a + 1) * HD)
                        nc.tensor.matmul(
                            sc[:, a * 512:(a + 1) * 512],
                            kT0[rows, j, st * P:(st + 1) * P],
                            qT0[rows, j, :],
                            start=True, stop=True,
                        )
                    else:
                        nc.tensor.matmul(
                            sc[:, a * 512:(a + 1) * 512],
                            k8s[:, :, h, ssl], q8s[:, :, h, tsl],
                            start=True, stop=True, perf_mode=DR,
                        )
                e = sb.tile([P, 1024], BF16, tag="e", bufs=32,
                            name=f"e{tch}{j}_{st}")
                nc.scalar.activation(e[:], sc[:], AF.Exp, scale=EXP_SCALE)
                ftt = fts[(tch, st // 4)]
                eng = nc.gpsimd if st in (3, 9, 14) else nc.vector
                for a in range(2):
                    half = slice(a * 512, (a + 1) * 512)
                    eng.tensor_mul(e[:, half], e[:, half],
                                   ftt[:, st % 4, :])
                ebank[u][st] = e

            def alloc_accs(u):
                accs[u] = tuple(
                    ps.tile([P, 4, HD + 1], F32, tag="uacc", bufs=2,
                            name=f"acc{u[0]}{u[1]}{a}")
                    for a in range(2)
                )

            def pv_step(u, st, first=None, last=None):
                tch, j = u
                first = (st == 0) if first is None else first
                last = (st == ST - 1) if last is None else last
                e = ebank[u].pop(st)
                for a in range(2):
                    acc = accs[u][a]
                    for c in range(4):
                        nc.tensor.matmul(
                            acc[:, c, :],
                            e[:, a * 512 + c * P:a * 512 + (c + 1) * P],
                            vsb[:, st, 2 * j + a, :],
                            start=(first and c == 0),
                            stop=(last and c == 3),
                            skip_group_check=True,
                        )

            def norm_unit(u, tail=False):
                for a in range(2):
                    acc = accs[u][a]
                    rc = sb.tile([P, 4, 1], F32, tag="rc", bufs=4,
                                 name=f"rc{u[0]}{u[1]}{a}")
                    nc.vector.reciprocal(rc[:], acc[:, :, HD:HD + 1])
                    for c in range(4):
                        if (u, c) not in poTs:
                            poTs[(u, c)] = sb.tile(
                                [P, P], BF16, tag="pt", bufs=8,
                                name=f"pt{u[0]}{u[1]}{c}")
                        if tail and a == 1:
                            nc.scalar.mul(
                                poTs[(u, c)][:, a * HD:(a + 1) * HD],
                                acc[:, c, 0:HD], rc[:, c, :],
                            )
                        else:
                            nc.vector.tensor_scalar_mul(
                                poTs[(u, c)][:, a * HD:(a + 1) * HD],
                                acc[:, c, 0:HD], rc[:, c, :],
                            )

            def tp_one(u, c):
                tch, j = u
                tpp = ps.tile([P, P], BF16, tag="aux", bufs=2,
                              name=f"tpp{tch}{j}{c}")
                nc.tensor.transpose(tpp[:], poTs.pop((u, c))[:], ident[:])
                nc.vector.tensor_copy(
                    po[:, j, tch * 512 + c * P: tch * 512 + (c + 1) * P],
                    tpp[:],
                )

            def tp_unit(u):
                for c in range(4):
                    tp_one(u, c)

            def fin_one(tch, c, tag="aux"):
                """Out-proj for one t-block: both 512-wide d-chunks, one store."""
                tsl = slice(tch * 512 + c * P, tch * 512 + (c + 1) * P)
                for dch in range(2):
                    fptag = tag if dch == 0 else ("uacc" if tch == 3 else tag)
                    fp = ps.tile([P, 512], F32, tag=fptag, bufs=2,
                                 name=f"fp{tch}{c}{dch}")
                    dsl = slice(dch * 512, (dch + 1) * 512)
                    for j in range(2):
                        nc.tensor.matmul(fp[:], po[:, j, tsl], wo[:, j, dsl],
                                         start=(j == 0), stop=(j == 1))
                    fo = sb.tile([P, 512], F32, tag="fo", bufs=4,
                                 name=f"fo{tch}{c}{dch}")
                    if tag == "uacc" and dch == 0:
                        nc.scalar.copy(fo[:], fp[:])
                    else:
                        nc.vector.tensor_copy(fo[:], fp[:])
                    nc.sync.dma_start(out_d[tsl, dsl], fo[:])

            # ================= PHASE A: QKV chunks + u00/u01/u10 scores ======
            u00, u01, u10, u11 = (0, 0), (0, 1), (1, 0), (1, 1)
            for u in (u00, u01, u10, u11):
                ebank[u] = {}
            alloc_accs(u00)

            # chunk 0
            qk_part(0, 0, 0)          # q0 m0 -> qT0
            qk_part(0, 1, 0)          # k0 m0 -> kT0
            dma_ft(0, 0)
            dma_ht(1)
            nc.sync.dma_start(bv[:], bv_d[:])
            nc.sync.dma_start(ident[:], id_d[:])
            for st in range(4):
                scores_step(u00, st)
            qk_cast0(0, 0)
            qk_resh(0, 0, 0)
            qk_cast0(1, 0)
            qk_resh(0, 1, 0)
            qk_part(0, 0, 1)
            qk_part(0, 1, 1)
            dma_ft(0, 1)
            for st in range(4):
                scores_step(u01, st)
            qk_cast0(0, 1)
            qk_resh(0, 0, 1)
            qk_cast0(1, 1)
            qk_resh(0, 1, 1)
            # chunks 1-3, serpentine: each chunk's k m-blocks are emitted
            # while the previous groups' exps are still queued, so the
            # bias+reshuffle chain hides; q(n>=1) and v are off-critical.
            qk_part(1, 1, 0)
            nc.sync.dma_start(wv[:], wvT_d.rearrange("(k p) r -> p k r", p=P))
            for n in range(1, 4):
                for st in range(4 * n, 4 * n + 4):
                    scores_step(u00, st)
                if n == 1:
                    qk_cast0(1, 0, n=1)
                    qk_resh(1, 1, 0)
                qk_part(n, 1, 1)
                if n == 1:
                    qk_cast0(1, 1, n=1)
                qk_resh(n, 1, 1)
                qk_part(n, 0, 0)
                qk_part(n, 0, 1)
                qk_resh(n, 0)         # q merged, off critical path
                if n < 3:
                    dma_ht(n + 1)
                    dma_ft(0, n + 1)
                dma_ft(1, n - 1)
                for st in range(4 * n, 4 * n + 4):
                    scores_step(u01, st)
                for st in range(4 * n - 4, 4 * n):
                    scores_step(u10, st)
                for s in range(4 * n - 4, 4 * n):
                    v_stile(s)
                for st in range(4 * n - 4, 4 * n):
                    pv_step(u00, st)
                if n < 3:
                    qk_part(n + 1, 1, 0)
                    qk_resh(n + 1, 1, 0)
                if n == 3:
                    dma_ft(1, 3)

            # phase-A tail: last v group + u00 finish
            for s in range(12, 16):
                v_stile(s)
            for st in range(12, 16):
                pv_step(u00, st)
            nc.sync.dma_start(wo[:], woT_d.rearrange("(m p) d -> p m d", p=P))

            norm_unit(u00)
            tp_unit(u00)

            # ================= PHASE B: weave ===============================
            fresh = [u11, (2, 0), (2, 1), (3, 0), (3, 1)]
            work = [(u10, st) for st in range(12, 16)]
            for w in fresh:
                ebank.setdefault(w, {})
                work += [(w, st) for st in range(ST)]

            # replay queue: (unit, st) in replay order; fresh units appended
            # as their scores complete
            RORD = [0, 1, 2, 4, 5, 6, 7, 8, 10, 11, 12, 13, 3, 9, 14, 15]
            replayq = [(u01, st) for st in RORD]
            replayq += [(u10, st) for st in RORD]
            for w in fresh:
                replayq += [(w, st) for st in RORD]

            # fins become available per tch once both units' tps are done
            finq = []
            tp_done = {u00: True}
            fins_emitted = set()

            ft_sched = {
                (fresh[0], 2): (2, 0), (fresh[0], 6): (2, 1),
                (fresh[0], 10): (2, 2), (fresh[0], 14): (2, 3),
                ((2, 0), 2): (3, 0), ((2, 0), 6): (3, 1),
                ((2, 0), 10): (3, 2), ((2, 0), 14): (3, 3),
            }

            rpi = 0
            sci = 0
            cool = 0
            for (w, st) in work:
                scores_step(w, st)
                sci += 1
                # splice replay PV steps; a step can only replay once its e
                # tile is produced.  After a unit finishes (norm+transpose),
                # pause splicing so the next unit's first PV does not block
                # the PE pipeline while PSUM accumulators drain.
                budget = 2
                if cool > 0:
                    cool -= 1
                    budget = 0
                while budget > 0 and rpi < len(replayq):
                    ru, rst = replayq[rpi]
                    if rst not in ebank.get(ru, {}):
                        break  # not scored yet
                    if rst == 0:
                        alloc_accs(ru)
                    pv_step(ru, rst)
                    budget -= 1
                    rpi += 1
                    if rst == ST - 1:
                        norm_unit(ru)
                        tp_unit(ru)
                        tp_done[ru] = True
                        cool = 2
                        tch = ru[0]
                        other = (tch, 1 - ru[1])
                        if tp_done.get(other) and tch not in fins_emitted:
                            fins_emitted.add(tch)
                            finq += [(tch, c) for c in range(4)]
                        break
                # one fin (both d-chunks) every few scores steps
                if finq and sci % 3 == 0:
                    ftch, fc = finq.pop(0)
                    fin_one(ftch, fc, tag="uacc" if ftch == 3 else "aux")
                if (w, st) in ft_sched:
                    dma_ft(*ft_sched[(w, st)])

            # ---- tail: drain remaining replays, then c-pipelined tp+fin ----
            while rpi < len(replayq):
                ru, rst = replayq[rpi]
                if rst == 0:
                    alloc_accs(ru)
                pv_step(ru, rst)
                rpi += 1
                if rst == ST - 1:
                    norm_unit(ru, tail=True)
                    tch = ru[0]
                    other = (tch, 1 - ru[1])
                    if tp_done.get(other) and tch not in fins_emitted:
                        fins_emitted.add(tch)
                        tp_done[ru] = True
                        for c in range(4):
                            tp_one(ru, c)
                            fin_one(tch, c, tag="uacc")
                    else:
                        tp_unit(ru)
                        tp_done[ru] = True
                while finq:
                    ftch, fc = finq.pop(0)
                    fin_one(ftch, fc, tag="uacc" if ftch == 3 else "aux")
            while finq:
                ftch, fc = finq.pop(0)
                fin_one(ftch, fc, tag="uacc" if ftch == 3 else "aux")

    return nc


_NC = None
_LAST_RESULT = None


def _get_nc():
    global _NC
    if _NC is None:
        _NC = build_bass()
        if not _NC.is_finalized():
            _NC.finalize()
    return _NC


def kernel(hidden_states, focused_attention, Wq, bq, Wk, bk, Wv, bv, Wo, bo):
    bf = ml_dtypes.bfloat16
    hT = [np.ascontiguousarray(hidden_states[b].T).astype(bf) for b in range(B)]
    fT = [np.ascontiguousarray(focused_attention[b].T).astype(bf) for b in range(B)]

    in_maps = []
    for c in range(N_CORES):
        b, g = divmod(c, 4)
        rows = slice(g * R, (g + 1) * R)
        in_maps.append({
            "hT": hT[b],
            "fT": fT[b],
            "wqT": np.ascontiguousarray((Wq[rows] * (SCALING * QSC)).T).astype(bf),
            "wkT": np.ascontiguousarray((Wk[rows] * KSC).T).astype(bf),
            "wvT": np.ascontiguousarray(Wv[rows].T).astype(bf),
            "woT": np.ascontiguousarray(Wo[:, rows].T).astype(bf),
            "bq": np.ascontiguousarray(
                (bq[rows] * (SCALING * QSC))[:, None]).astype(np.float32),
            "bk": np.ascontiguousarray((bk[rows] * KSC)[:, None]).astype(np.float32),
            "bv": np.ascontiguousarray(bv[rows][None, :]).astype(bf),
            "ident": np.eye(P, dtype=bf),
        })

    res = run_bass_kernel_spmd(_get_nc(), in_maps, list(range(N_CORES)))
    global _LAST_RESULT
    _LAST_RESULT = res
    out = np.zeros((B, T, D), dtype=np.float32)
    for c in range(N_CORES):
        out[c // 4] += res.results[c]["out_partial"]
    out += np.asarray(bo, dtype=np.float32)[None, None, :]
    return out


# revision 8
# speedup vs baseline: 1.0086x; 1.0018x over previous
"""BartAttention (focused-attention variant) Trainium2 Bass kernel, v2.

Problem (hardcoded): B=2, T=2048, D=1024, H=16 heads, hd=64.
  q = (h @ Wq.T + bq) * hd**-0.5 ; k = h @ Wk.T + bk ; v = h @ Wv.T + bv
  scores = q @ k.T per head ; e = f * exp(scores) ; attn = e / rowsum(e)
  out = (attn @ v) @ Wo.T + bo

Sharding over 8 cores: batch (2) x head-group (4 groups of 4 heads); host
sums the 4 partial out-projections per batch and adds bo.

Per-core design (4 heads = 2 pairs j=0,1; ACT-exp is the bottleneck engine
at ~134us busy, everything else is scheduled to hide under it):
  - q/k projections bf16; q,k pre-scaled x16/x8 on the host and stored as
    fp8e4m3 in an hd-split [32 partitions, half, head, t] layout via
    SBUF->SBUF reshuffle DMAs
  - scores: fp8 DoubleRow matmuls (two 32-row contraction tiles per
    instruction -> 256 cycles per [128,512] block, 2x over bf16);
    sc = 128*q.k in PSUM f32; the first 4 s-tile groups of the (0,j) units
    use a bf16 path instead so the exp stream starts before any reshuffle
  - exp: ACT only, e = exp(sc/128) on [128,1024] tiles (2 heads);
    e *= fT in place on DVE (a few steps per unit go to GPSIMD to keep
    DVE under ACT)
  - PV e-stationary: acc[t-block, 65] += e_block.T @ [v|1]; N=65 matmuls;
    column 64 accumulates rowsum(e), so normalization is a per-partition
    reciprocal + tensor_scalar multiply (no broadcast matmuls)
  - po blocks transposed to [r, t] with PE transpose + DVE copy;
    out-proj per (t-block, d-chunk) accumulates both head pairs, DVE/ACT
    copies PSUM->SBUF, DMA out
  - schedule: QKV chunks serpentined with the scores of units (0,0), (0,1)
    and (1,0) (PV deferred into an SBUF e-backlog) so ACT never starves in
    phase A; phase B replays each unit's PV one unit behind the fresh
    scores stream, with out-projections and transposes woven between.
"""

import numpy as np
import ml_dtypes

import concourse.bass as bass
import concourse.bacc as bacc
import concourse.mybir as mybir
from concourse.tile import TileContext
from concourse.bass_utils import run_bass_kernel_spmd

BF16 = mybir.dt.bfloat16
F32 = mybir.dt.float32
F8 = mybir.dt.float8e4
AF = mybir.ActivationFunctionType
DR = mybir.MatmulPerfMode.DoubleRow

B, T, D = 2, 2048, 1024
H, HD = 16, 64
HG = 4               # heads per core
R = HG * HD          # 256 rows per core
SCALING = HD ** -0.5
N_CORES = 8

P = 128
KT = D // P          # 8 k-tiles for QKV contraction
NCH = T // 512       # 4 t-chunks
ST = T // P          # 16 s-tiles

QSC, KSC = 16.0, 8.0         # fp8 pre-scales for q and k
EXP_SCALE = 1.0 / (QSC * KSC)


def build_bass():
    nc = bacc.Bacc()

    hT_d = nc.declare_dram_parameter("hT", [D, T], BF16, isOutput=False)
    fT_d = nc.declare_dram_parameter("fT", [T, T], BF16, isOutput=False)
    wqT_d = nc.declare_dram_parameter("wqT", [D, R], BF16, isOutput=False)
    wkT_d = nc.declare_dram_parameter("wkT", [D, R], BF16, isOutput=False)
    wvT_d = nc.declare_dram_parameter("wvT", [D, R], BF16, isOutput=False)
    woT_d = nc.declare_dram_parameter("woT", [R, D], BF16, isOutput=False)
    bq_d = nc.declare_dram_parameter("bq", [R, 1], F32, isOutput=False)
    bk_d = nc.declare_dram_parameter("bk", [R, 1], F32, isOutput=False)
    bv_d = nc.declare_dram_parameter("bv", [1, R], BF16, isOutput=False)
    id_d = nc.declare_dram_parameter("ident", [P, P], BF16, isOutput=False)
    out_d = nc.declare_dram_parameter("out_partial", [T, D], F32, isOutput=True)

    hT_r = hT_d.rearrange("(k p) t -> p k t", p=P)
    fT_r = fT_d.rearrange("(s p) t -> p s t", p=P)

    with TileContext(nc) as tc:
        with (
            nc.allow_low_precision(reason="bf16/fp8 pipeline is intentional"),
            tc.tile_pool(name="sb", bufs=1) as sb,
            tc.tile_pool(name="ps", bufs=1, space="PSUM") as ps,
        ):
            # ---- persistent SBUF ----
            wq = sb.tile([P, KT, R], BF16)
            wk = sb.tile([P, KT, R], BF16)
            wv = sb.tile([P, KT, R], BF16)
            wo = sb.tile([P, 2, D], BF16)
            bq = sb.tile([P, 2], F32)
            bk = sb.tile([P, 2], F32)
            bv = sb.tile([1, R], BF16)
            ident = sb.tile([P, P], BF16)
            ones_r = sb.tile([1, P], BF16)
            q8s = sb.tile([32, 2, HG, T], F8)   # [p32, half, head, t]
            k8s = sb.tile([32, 2, HG, T], F8)
            vsb = sb.tile([P, ST, HG, HD + 1], BF16)
            qT0 = sb.tile([P, 2, 512], BF16)    # chunk-0 q bf16 (ramp path)
            kT0 = sb.tile([P, 2, 1024], BF16)   # chunks 0-1 k bf16 (seam path)
            po = sb.tile([P, 2, T], BF16)       # out-proj lhsT [r, (tch c t)]

            nc.vector.memset(ones_r[:], 1.0)
            nc.vector.memset(vsb[:, :, :, HD:HD + 1], 1.0)

            # PE warm-up: burn the p-state ramp on junk matmuls during DMA
            for i in range(24):
                wrm = ps.tile([P, P], F32, tag="sc", bufs=2, name=f"warm{i}")
                nc.tensor.matmul(wrm[:], ones_r[:], ones_r[:],
                                 start=True, stop=True)

            # ---- initial DMAs ----
            wq_r = wqT_d.rearrange("(k p) r -> p k r", p=P)
            wk_r = wkT_d.rearrange("(k p) r -> p k r", p=P)
            nc.sync.dma_start(wq[:, 0:4, :], wq_r[:, 0:4, :])
            ht = {}

            def dma_ht(n):
                t = sb.tile([P, KT, 512], BF16, tag="ht", bufs=3, name=f"ht{n}")
                nsl = slice(n * 512, (n + 1) * 512)
                nc.sync.dma_start(t[:, 0:4, :], hT_r[:, 0:4, nsl])
                nc.sync.dma_start(t[:, 4:8, :], hT_r[:, 4:8, nsl])
                ht[n] = t

            ht0 = sb.tile([P, KT, 512], BF16, tag="ht", bufs=3, name="ht0")
            nc.sync.dma_start(ht0[:, 0:4, :], hT_r[:, 0:4, 0:512])
            nc.sync.dma_start(wk[:, 0:4, :], wk_r[:, 0:4, :])
            nc.sync.dma_start(wq[:, 4:8, :], wq_r[:, 4:8, :])
            nc.sync.dma_start(ht0[:, 4:8, :], hT_r[:, 4:8, 0:512])
            nc.sync.dma_start(wk[:, 4:8, :], wk_r[:, 4:8, :])
            ht[0] = ht0
            nc.sync.dma_start(bq[:], bq_d.rearrange("(m p) one -> p (m one)", p=P))
            nc.sync.dma_start(bk[:], bk_d.rearrange("(m p) one -> p (m one)", p=P))

            fts = {}

            def dma_ft(tch, g):
                t = sb.tile([P, 4, 512], BF16, tag="ft", bufs=8,
                            name=f"ft{tch}g{g}")
                nc.sync.dma_start(
                    t[:], fT_r[:, 4 * g:4 * g + 4, tch * 512:(tch + 1) * 512]
                )
                fts[(tch, g)] = t

            # ---- helpers ----
            st8s = {}

            def qk_part(n, tens, m, cols=None):
                """q or k projection matmuls + bias for chunk n, m-block.
                cols: optional (lo, hi) sub-range of the 512 chunk columns."""
                w_sb, b_sb = (wq, bq) if tens == 0 else (wk, bk)
                lo, hi = cols if cols else (0, 512)
                nm = f"{'qk'[tens]}{n}m{m}c{lo}"
                acc = ps.tile([P, hi - lo], F32, tag="aux", bufs=2,
                              name=f"a{nm}")
                for kk in range(KT):
                    nc.tensor.matmul(
                        acc[:], w_sb[:, kk, m * P:(m + 1) * P],
                        ht[n][:, kk, lo:hi],
                        start=(kk == 0), stop=(kk == KT - 1),
                    )
                if n == 0 or (n == 1 and tens == 1):
                    dst = qT0 if tens == 0 else kT0
                    off = 0 if n == 0 else 512
                    nc.vector.tensor_scalar_add(dst[:, m, off + lo:off + hi],
                                                acc[:], b_sb[:, m:m + 1])
                    return
                if (tens, n) not in st8s:
                    st8s[(tens, n)] = sb.tile([P, 2, 512], F8, tag="st8",
                                              bufs=4, name=f"s{'qk'[tens]}{n}")
                nc.vector.tensor_scalar_add(st8s[(tens, n)][:, m, lo:hi],
                                            acc[:], b_sb[:, m:m + 1])

            def qk_cast0(tens, m, n=0):
                if (tens, n) not in st8s:
                    st8s[(tens, n)] = sb.tile([P, 2, 512], F8, tag="st8",
                                              bufs=4, name=f"s{'qk'[tens]}{n}")
                src_t = qT0 if tens == 0 else kT0
                off = 0 if n == 0 else 512
                nc.vector.tensor_copy(st8s[(tens, n)][:, m, :],
                                      src_t[:, m, off:off + 512])

            def qk_resh(n, tens, m=None, cols=None):
                """Reshuffle chunk n into the [32, half, head, t] hd-split
                layout. m=None: both m-blocks in 4 DMAs of [32, 2, 512]
                (m via stride-2 head dim); m=int: that m-block only."""
                dst = q8s if tens == 0 else k8s
                st8 = st8s[(tens, n)]
                lo, hi = cols if cols else (0, 512)
                nsl = slice(n * 512 + lo, n * 512 + hi)
                for half in range(2):
                    for hm in range(2):
                        src_p = slice(64 * hm + 32 * half, 64 * hm + 32 * half + 32)
                        if m is None:
                            nc.sync.dma_start(dst[:, half, hm::2, nsl],
                                              st8[src_p, :, lo:hi])
                        else:
                            nc.sync.dma_start(dst[:, half, 2 * m + hm, nsl],
                                              st8[src_p, m, lo:hi])

            def v_stile(s):
                acc = ps.tile([P, R], F32, tag="aux", bufs=2, name=f"vacc{s}")
                for kk in range(KT):
                    nc.tensor.matmul(
                        acc[:], ht[s // 4][:, kk, (s % 4) * P:(s % 4 + 1) * P],
                        wv[:, kk, :], start=(kk == 0), stop=False,
                    )
                nc.tensor.matmul(acc[:], ones_r[:], bv[:], start=False, stop=True)
                nc.vector.tensor_copy(
                    vsb[:, s, :, 0:HD],
                    acc[:].rearrange("p (h d) -> p h d", h=HG),
                )

            ebank = {}    # u -> {st: e_tile} pending PV
            accs = {}     # u -> (acc_a, acc_b)
            poTs = {}     # (u, c) -> poT tile

            def scores_step(u, st):
                tch, j = u
                sc = ps.tile([P, 1024], F32, tag="sc", bufs=2,
                             name=f"sc{tch}{j}_{st}")
                tsl = slice(tch * 512, (tch + 1) * 512)
                ssl = slice(st * P, (st + 1) * P)
                for a in range(2):
                    h = 2 * j + a
                    if tch == 0 and st < 8:
                        rows = slice(a * HD, (a + 1) * HD)
                        nc.tensor.matmul(
                            sc[:, a * 512:(a + 1) * 512],
                            kT0[rows, j, st * P:(st + 1) * P],
                            qT0[rows, j, :],
                            start=True, stop=True,
                        )
                    else:
                        nc.tensor.matmul(
                            sc[:, a * 512:(a + 1) * 512],
                            k8s[:, :, h, ssl], q8s[:, :, h, tsl],
                            start=True, stop=True, perf_mode=DR,
                        )
                e = sb.tile([P, 1024], BF16, tag="e", bufs=32,
                            name=f"e{tch}{j}_{st}")
                nc.scalar.activation(e[:], sc[:], AF.Exp, scale=EXP_SCALE)
                ftt = fts[(tch, st // 4)]
                eng = nc.gpsimd if st in (3, 9, 14) else nc.vector
                for a in range(2):
                    half = slice(a * 512, (a + 1) * 512)
                    eng.tensor_mul(e[:, half], e[:, half],
                                   ftt[:, st % 4, :])
                ebank[u][st] = e

            def alloc_accs(u):
                accs[u] = tuple(
                    ps.tile([P, 4, HD + 1], F32, tag="uacc", bufs=2,
                            name=f"acc{u[0]}{u[1]}{a}")
                    for a in range(2)
                )

            def pv_step(u, st, first=None, last=None):
                tch, j = u
                first = (st == 0) if first is None else first
                last = (st == ST - 1) if last is None else last
                e = ebank[u].pop(st)
                for a in range(2):
                    acc = accs[u][a]
                    for c in range(4):
                        nc.tensor.matmul(
                            acc[:, c, :],
                            e[:, a * 512 + c * P:a * 512 + (c + 1) * P],
                            vsb[:, st, 2 * j + a, :],
                            start=(first and c == 0),
                            stop=(last and c == 3),
                            skip_group_check=True,
                        )

            def norm_unit(u, tail=False):
                rcs = []
                for a in range(2):
                    rc = sb.tile([P, 4, 1], F32, tag="rc", bufs=4,
                                 name=f"rc{u[0]}{u[1]}{a}")
                    nc.vector.reciprocal(rc[:], accs[u][a][:, :, HD:HD + 1])
                    rcs.append(rc)
                for c in range(4):
                    if (u, c) not in poTs:
                        poTs[(u, c)] = sb.tile(
                            [P, P], BF16, tag="pt", bufs=8,
                            name=f"pt{u[0]}{u[1]}{c}")
                    for a in range(2):
                        nc.vector.tensor_scalar_mul(
                            poTs[(u, c)][:, a * HD:(a + 1) * HD],
                            accs[u][a][:, c, 0:HD], rcs[a][:, c, :],
                        )

            def tp_one(u, c):
                tch, j = u
                tpp = ps.tile([P, P], BF16, tag="aux", bufs=2,
                              name=f"tpp{tch}{j}{c}")
                nc.tensor.transpose(tpp[:], poTs.pop((u, c))[:], ident[:])
                nc.vector.tensor_copy(
                    po[:, j, tch * 512 + c * P: tch * 512 + (c + 1) * P],
                    tpp[:],
                )

            def tp_unit(u):
                for c in range(4):
                    tp_one(u, c)

            def fin_one(tch, c, tag="aux"):
                """Out-proj for one t-block: both 512-wide d-chunks, one store."""
                tsl = slice(tch * 512 + c * P, tch * 512 + (c + 1) * P)
                for dch in range(2):
                    fptag = tag if dch == 0 else ("uacc" if tch == 3 else tag)
                    fp = ps.tile([P, 512], F32, tag=fptag, bufs=2,
                                 name=f"fp{tch}{c}{dch}")
                    dsl = slice(dch * 512, (dch + 1) * 512)
                    for j in range(2):
                        nc.tensor.matmul(fp[:], po[:, j, tsl], wo[:, j, dsl],
                                         start=(j == 0), stop=(j == 1))
                    fo = sb.tile([P, 512], F32, tag="fo", bufs=4,
                                 name=f"fo{tch}{c}{dch}")
                    if tag == "uacc" and dch == 0:
                        nc.scalar.copy(fo[:], fp[:])
                    else:
                        nc.vector.tensor_copy(fo[:], fp[:])
                    nc.sync.dma_start(out_d[tsl, dsl], fo[:])

            # ================= PHASE A: QKV chunks + u00/u01/u10 scores ======
            u00, u01, u10, u11 = (0, 0), (0, 1), (1, 0), (1, 1)
            for u in (u00, u01, u10, u11):
                ebank[u] = {}
            alloc_accs(u00)

            # chunk 0
            qk_part(0, 0, 0)          # q0 m0 -> qT0
            qk_part(0, 1, 0)          # k0 m0 -> kT0
            dma_ft(0, 0)
            dma_ht(1)
            nc.sync.dma_start(bv[:], bv_d[:])
            nc.sync.dma_start(ident[:], id_d[:])
            for st in range(4):
                scores_step(u00, st)
            qk_cast0(0, 0)
            qk_resh(0, 0, 0)
            qk_cast0(1, 0)
            qk_resh(0, 1, 0)
            qk_part(0, 0, 1)
            qk_part(0, 1, 1)
            dma_ft(0, 1)
            for st in range(4):
                scores_step(u01, st)
            qk_cast0(0, 1)
            qk_resh(0, 0, 1)
            qk_cast0(1, 1)
            qk_resh(0, 1, 1)
            # chunks 1-3, serpentine: each chunk's k m-blocks are emitted
            # while the previous groups' exps are still queued, so the
            # bias+reshuffle chain hides; q(n>=1) and v are off-critical.
            qk_part(1, 1, 0)
            nc.sync.dma_start(wv[:], wvT_d.rearrange("(k p) r -> p k r", p=P))
            for n in range(1, 4):
                for st in range(4 * n, 4 * n + 4):
                    scores_step(u00, st)
                if n == 1:
                    qk_cast0(1, 0, n=1)
                    qk_resh(1, 1, 0)
                qk_part(n, 1, 1)
                if n == 1:
                    qk_cast0(1, 1, n=1)
                qk_resh(n, 1, 1)
                qk_part(n, 0, 0)
                qk_part(n, 0, 1)
                qk_resh(n, 0)         # q merged, off critical path
                if n < 3:
                    dma_ht(n + 1)
                    dma_ft(0, n + 1)
                dma_ft(1, n - 1)
                for st in range(4 * n, 4 * n + 4):
                    scores_step(u01, st)
                for st in range(4 * n - 4, 4 * n):
                    scores_step(u10, st)
                for s in range(4 * n - 4, 4 * n):
                    v_stile(s)
                for st in range(4 * n - 4, 4 * n):
                    pv_step(u00, st)
                if n < 3:
                    qk_part(n + 1, 1, 0)
                    qk_resh(n + 1, 1, 0)
                if n == 3:
                    dma_ft(1, 3)

            # phase-A tail: last v group + u00 finish
            for s in range(12, 16):
                v_stile(s)
            for st in range(12, 16):
                pv_step(u00, st)
            nc.sync.dma_start(wo[:], woT_d.rearrange("(m p) d -> p m d", p=P))

            norm_unit(u00)
            tp_unit(u00)

            # ================= PHASE B: weave ===============================
            fresh = [u11, (2, 0), (2, 1), (3, 0), (3, 1)]
            work = [(u10, st) for st in range(12, 16)]
            for w in fresh:
                ebank.setdefault(w, {})
                work += [(w, st) for st in range(ST)]

            # replay queue: (unit, st) in replay order; fresh units appended
            # as their scores complete
            RORD = [0, 1, 2, 4, 5, 6, 7, 8, 10, 11, 12, 13, 3, 9, 14, 15]
            replayq = [(u01, st) for st in RORD]
            replayq += [(u10, st) for st in RORD]
            for w in fresh:
                replayq += [(w, st) for st in RORD]

            # fins become available per tch once both units' tps are done
            finq = []
            tp_done = {u00: True}
            fins_emitted = set()

            ft_sched = {
                (fresh[0], 2): (2, 0), (fresh[0], 6): (2, 1),
                (fresh[0], 10): (2, 2), (fresh[0], 14): (2, 3),
                ((2, 0), 2): (3, 0), ((2, 0), 6): (3, 1),
                ((2, 0), 10): (3, 2), ((2, 0), 14): (3, 3),
            }

            rpi = 0
            sci = 0
            cool = 0
            for (w, st) in work:
                scores_step(w, st)
                sci += 1
                # splice replay PV steps; a step can only replay once its e
                # tile is produced.  After a unit finishes (norm+transpose),
                # pause splicing so the next unit's first PV does not block
                # the PE pipeline while PSUM accumulators drain.
                budget = 2
                if cool > 0:
                    cool -= 1
                    budget = 0
                while budget > 0 and rpi < len(replayq):
                    ru, rst = replayq[rpi]
                    if rst not in ebank.get(ru, {}):
                        break  # not scored yet
                    if rst == 0:
                        alloc_accs(ru)
                    pv_step(ru, rst)
                    budget -= 1
                    rpi += 1
                    if rst == ST - 1:
                        norm_unit(ru)
                        tp_unit(ru)
                        tp_done[ru] = True
                        cool = 2
                        tch = ru[0]
                        other = (tch, 1 - ru[1])
                        if tp_done.get(other) and tch not in fins_emitted:
                            fins_emitted.add(tch)
                            finq += [(tch, c) for c in range(4)]
                        break
                # one fin (both d-chunks) every few scores steps
                if finq and sci % 3 == 0:
                    ftch, fc = finq.pop(0)
                    fin_one(ftch, fc, tag="uacc" if ftch == 3 else "aux")
                if (w, st) in ft_sched:
                    dma_ft(*ft_sched[(w, st)])

            # ---- tail: drain remaining replays, then c-pipelined tp+fin ----
            while rpi < len(replayq):
                ru, rst = replayq[rpi]
                if rst == 0:
                    alloc_accs(ru)
                pv_step(ru, rst)
                rpi += 1
                if rst == ST - 1:
                    norm_unit(ru, tail=True)
                    tch = ru[0]
                    other = (tch, 1 - ru[1])
                    if tp_done.get(other) and tch not in fins_emitted:
                        fins_emitted.add(tch)
                        tp_done[ru] = True
                        for c in range(4):
                            tp_one(ru, c)
                            fin_one(tch, c, tag="uacc")
                    else:
                        tp_unit(ru)
                        tp_done[ru] = True
                while finq:
                    ftch, fc = finq.pop(0)
                    fin_one(ftch, fc, tag="uacc" if ftch == 3 else "aux")
            while finq:
                ftch, fc = finq.pop(0)
                fin_one(ftch, fc, tag="uacc" if ftch == 3 else "aux")

    return nc


_NC = None
_LAST_RESULT = None


def _get_nc():
    global _NC
    if _NC is None:
        _NC = build_bass()
        if not _NC.is_finalized():
            _NC.finalize()
    return _NC


def kernel(hidden_states, focused_attention, Wq, bq, Wk, bk, Wv, bv, Wo, bo):
    bf = ml_dtypes.bfloat16
    hT = [np.ascontiguousarray(hidden_states[b].T).astype(bf) for b in range(B)]
    fT = [np.ascontiguousarray(focused_attention[b].T).astype(bf) for b in range(B)]

    in_maps = []
    for c in range(N_CORES):
        b, g = divmod(c, 4)
        rows = slice(g * R, (g + 1) * R)
        in_maps.append({
            "hT": hT[b],
            "fT": fT[b],
            "wqT": np.ascontiguousarray((Wq[rows] * (SCALING * QSC)).T).astype(bf),
            "wkT": np.ascontiguousarray((Wk[rows] * KSC).T).astype(bf),
            "wvT": np.ascontiguousarray(Wv[rows].T).astype(bf),
            "woT": np.ascontiguousarray(Wo[:, rows].T).astype(bf),
            "bq": np.ascontiguousarray(
                (bq[rows] * (SCALING * QSC))[:, None]).astype(np.float32),
            "bk": np.ascontiguousarray((bk[rows] * KSC)[:, None]).astype(np.float32),
            "bv": np.ascontiguousarray(bv[rows][None, :]).astype(bf),
            "ident": np.eye(P, dtype=bf),
        })

    res = run_bass_kernel_spmd(_get_nc(), in_maps, list(range(N_CORES)))
    global _LAST_RESULT
    _LAST_RESULT = res
    out = np.zeros((B, T, D), dtype=np.float32)
    for c in range(N_CORES):
        out[c // 4] += res.results[c]["out_partial"]
    out += np.asarray(bo, dtype=np.float32)[None, None, :]
    return out
